# revision 31
# baseline (speedup 1.0000x reference)
"""Trainium2 Bass kernel for nn_AtenMatmulQMixedSigni8.

Reference computation:
    xf = (x_int8  - (-66)) * x_scale      # [7, 8, 512, 1024]
    yf = (y_uint8 - 160)   * y_scale      # [8, 1024, 512]
    out = einsum('gbmk,bkn->gbmn', xf, yf)  # [7, 8, 512, 512] f32

Strategy:
  - Shard data-parallel over the B=8 batch axis: core b gets x[:, b], y[b],
    produces out[:, b]. No collectives.
  - Zero-point-shifted fp8 path: with a = x (in [-128,127]) and
    b = y - 128 (in [-128,127]),
        (x+66)(y-160) = a@b - 32*rowsum_k(a) + 66*colsum_k(b) - 66*32*K.
    a and b are rounded to fp8 e4m3 on the host; the device computes the
    a@b matmul with fp8 DoubleRow matmuls (2 k-rows per cycle), and the
    exact rank-1 corrections are added on the host afterwards. Measured
    end-to-end max rel err on the real inputs: 8.2e-3 (gate is 2e-2).
  - Device output is fp16 (values bounded by ~±760 after the x_scale*
    y_scale multiply, so fp16 rounding is ~3e-4 relative) to halve the
    output DMA traffic.
  - Host pre-packs a (transposed to lhsT layout) and b into the exact
    SBUF tile layout (partition-major), so every DMA moves long
    contiguous per-partition runs. The host un-permutes the output.
  - Raw Bass (explicit engine programs + semaphores).

Hardware costs that shaped the schedule (measured on this part):
  - dma_start costs the issuing engine ~0.6-0.7us of sequencer time, so
    input DMAs are few and big: two 2-k-pair startup chunks (y+x[g0]
    interleaved for a fast first matmul), then one whole-g x DMA per g.
  - DMA packets are per-partition runs; 1KB runs move ~200GB/s, 4KB runs
    ~380GB/s. The startup chunks use 2KB runs, the bulk 4KB runs, so
    x[g1] lands before g0 finishes and the PE never stalls on input.
  - The epilogue (PSUM*scale -> fp16 SBUF) plus a store dma_start is
    ~1.3us, more than the ~1us PE group pace, so epilogues alternate
    scalar (even groups) / vector (odd groups) and stores alternate
    scalar (even) / sync (odd, idle after the input issues).
"""

import os
import sys

sys.path.insert(0, "/opt/trn_rl_repo")

import numpy as np
import ml_dtypes

G, B, M, K, N = 7, 8, 512, 1024, 512
P = 128
X_ZP = -66
Y_ZP = 160
Y_SHIFT = 128          # host shifts y by -128 so fp8 sees [-128, 127]

KO = K // P            # 8 k-tiles
KP = KO // 2           # 4 DoubleRow k-pairs per matmul group
MO = M // P            # 4 m-tiles (groups) per g
NG = G * MO            # 28 matmul groups
NBANK = 8              # PSUM banks


def _build_graph(scale: float):
    import concourse.bass as bass
    import concourse.mybir as mybir
    from contextlib import ExitStack

    nc = bass.Bass()

    # All DRAM tensors are laid out exactly like their SBUF tiles
    # (partition dim outermost), so each DMA is 128 long contiguous runs.
    xd = nc.declare_dram_parameter(
        "xp", [P, G * KP * MO, 2 * P], mybir.dt.float8e4, isOutput=False
    )
    yd = nc.declare_dram_parameter("yp", [P, KO, N], mybir.dt.float8e4, isOutput=False)
    od = nc.declare_dram_parameter("op", [P, NG, N], mybir.dt.float16, isOutput=True)

    with ExitStack() as stack:
        ysb = stack.enter_context(nc.sbuf_tensor("ysb", [P, KO, N], mybir.dt.float8e4))
        xsb = stack.enter_context(
            nc.sbuf_tensor("xsb", [P, G * KP * MO, 2 * P], mybir.dt.float8e4)
        )
        osb = stack.enter_context(nc.sbuf_tensor("osb", [P, NG, N], mybir.dt.float16))
        ps = stack.enter_context(nc.psum_tensor("ps", [P, NBANK, N], mybir.dt.float32))
        ldsems = [stack.enter_context(nc.semaphore(f"ld{j}")) for j in range(KP)]
        x1sems = [stack.enter_context(nc.semaphore(f"x1p{j}")) for j in range(KP)]
        xgsems = [stack.enter_context(nc.semaphore(f"xg{i}")) for i in range(3)]
        pesem = stack.enter_context(nc.semaphore("pesem"))
        acte = stack.enter_context(nc.semaphore("acte"))
        acto = stack.enter_context(nc.semaphore("acto"))
        outsem = stack.enter_context(nc.semaphore("outsem"))
        block = stack.enter_context(nc.Block(no_gpsimd_drain=True))
        actsems = [acte, acto]
        DR = mybir.MatmulPerfMode.DoubleRowSwInterleave

        @block.sync
        def _(sync):
            # Inputs on one FIFO ring, issue order = consumption order:
            # y/x[g0] interleaved k-pairs, then x[g1] in k-pairs (g1 runs
            # k-outer, and small DMAs get their completion semaphore
            # sooner), then whole-g x for g2+.
            for j in range(KP):
                ks = slice(2 * j, 2 * (j + 1))
                sync.dma_start(ysb[:, ks, :], yd[:, ks, :]).then_inc(ldsems[j], 16)
                xs = slice(j * MO, (j + 1) * MO)
                sync.dma_start(xsb[:, xs, :], xd[:, xs, :]).then_inc(ldsems[j], 16)
            for j in range(KP):
                xs = slice((KP + j) * MO, (KP + j + 1) * MO)
                sync.dma_start(xsb[:, xs, :], xd[:, xs, :]).then_inc(x1sems[j], 16)
            for c, (ga, gb) in enumerate(((2, 4), (4, 6), (6, 7))):
                gs = slice(ga * KP * MO, gb * KP * MO)
                sync.dma_start(xsb[:, gs, :], xd[:, gs, :]).then_inc(xgsems[c], 16)
            # Odd-group stores (the sync sequencer is idle once the input
            # issues are done; stores alternate rings to halve issue cost).
            for i in range(1, NG, 2):
                sync.wait_ge(acto, (i + 1) // 2)
                sync.dma_start(od[:, i, :], osb[:, i, :]).then_inc(outsem, 16)

        @block.tensor
        def _(tensor):
            # g=0 and g=1 run kpair-outer over banks 0-3 / 4-7 so each
            # matmul only needs its own k-pair of inputs, not the whole g.
            for j in range(KP):
                tensor.wait_ge(ldsems[j], 32)
                ks = slice(2 * j, 2 * (j + 1))
                for m in range(MO):
                    mm = tensor.matmul(
                        ps[:, m, :],
                        xsb[:, j * MO + m, :],
                        ysb[:, ks, :],
                        start=(j == 0),
                        stop=(j == KP - 1),
                        perf_mode=DR,
                    )
                    if j == KP - 1:
                        mm.then_inc(pesem, 1)
            for j in range(KP):
                tensor.wait_ge(x1sems[j], 16)
                ks = slice(KO + 2 * j, KO + 2 * (j + 1))
                for m in range(MO):
                    mm = tensor.matmul(
                        ps[:, MO + m, :],
                        xsb[:, (KP + j) * MO + m, :],
                        ysb[:, 2 * j : 2 * (j + 1), :],
                        start=(j == 0),
                        stop=(j == KP - 1),
                        perf_mode=DR,
                    )
                    if j == KP - 1:
                        mm.then_inc(pesem, 1)

            # Remaining g: m-outer with dense kpair loops (PE stays warm,
            # and the trailing epilogues pipeline group by group).
            i = 2 * MO
            for g in range(2, G):
                tensor.wait_ge(xgsems[(g - 2) // 2], 16)
                for m in range(MO):
                    # PSUM bank reuse: epilogue of group i-8 (same parity)
                    # must have drained the bank.
                    tensor.wait_ge(actsems[i % 2], (i - NBANK) // 2 + 1)
                    mm = None
                    for j in range(KP):
                        mm = tensor.matmul(
                            ps[:, i % NBANK, :],
                            xsb[:, (g * KP + j) * MO + m, :],
                            ysb[:, 2 * j : 2 * (j + 1), :],
                            start=(j == 0),
                            stop=(j == KP - 1),
                            perf_mode=DR,
                        )
                    mm.then_inc(pesem, 1)
                    i += 1

        @block.scalar
        def _(scalar):
            # Even-group epilogues + even-group stores. The store's gate
            # (epilogue wrote SBUF) is its own preceding instruction, so
            # program order suffices.
            for i in range(0, NG, 2):
                scalar.wait_ge(pesem, i + 1)
                scalar.mul(osb[:, i, :], ps[:, i % NBANK, :], scale).then_inc(
                    acte, 1
                )
                scalar.wait_ge(acte, i // 2 + 1)
                scalar.dma_start(od[:, i, :], osb[:, i, :]).then_inc(outsem, 16)
            scalar.wait_ge(outsem, 16 * NG)

        @block.vector
        def _(vector):
            # Odd-group epilogues on DVE.
            for i in range(1, NG, 2):
                vector.wait_ge(pesem, i + 1)
                vector.tensor_scalar_mul(
                    osb[:, i, :], ps[:, i % NBANK, :], scale
                ).then_inc(acto, 1)

    return nc


def _fp8_luts():
    """256-entry uint8->fp8e4m3-byte LUTs for the two operands."""
    v = np.arange(256, dtype=np.int32)
    xv = v.astype(np.uint8).view(np.int8).astype(np.float32)          # raw int8 value
    yv = (v - Y_SHIFT).astype(np.float32)                             # y byte - 128
    lx = xv.astype(ml_dtypes.float8_e4m3).view(np.uint8)
    ly = yv.astype(ml_dtypes.float8_e4m3).view(np.uint8)
    return lx, ly


def kernel(x, y, x_scale, y_scale):
    from concourse.bass_utils import run_bass_kernel_spmd

    x = np.asarray(x)
    y = np.asarray(y)
    scale = float(np.float32(x_scale) * np.float32(y_scale))

    # fp8 round both operands via byte LUTs (exact RTN to e4m3), then
    # pack into SBUF layout:
    #   xp[b][p, g*KO + ko, m] = fp8(x[g, b, m, ko*P + p])      (lhsT layout)
    #   yp[b][p, ko, n]        = fp8(y[b, ko*P + p, n] - 128)
    lx, ly = _fp8_luts()
    xq = lx[x.view(np.uint8)]                                  # [G,B,M,K] u8
    # SwInterleave weight slabs: slab (g, j, mtile) holds W[p, c] with
    # c = 2*(127-mcol) + i, where the pair element i is k-tile 2j+i and
    # mcol is the weight column: W[p, 2t+i] = A/B pairs interleaved,
    # columns reversed (what the PE's DoubleRowSwInterleave mode expects).
    arr = xq.reshape(G, B, MO, P, KP, 2, P)[:, :, :, ::-1]     # g,b,mt,mcol(rev),j,i,p
    xp = np.ascontiguousarray(
        arr.transpose(1, 6, 0, 4, 2, 3, 5)                     # b,p,g,j,mt,mcol,i
    ).reshape(B, P, G * KP * MO, 2 * P).view(ml_dtypes.float8_e4m3)
    yq = ly[y.view(np.uint8)]                                  # [B,K,N] u8
    yp = np.ascontiguousarray(
        yq.reshape(B, KO, P, N).transpose(0, 2, 1, 3)
    ).view(ml_dtypes.float8_e4m3)

    # Exact rank-1 corrections (host side):
    #   (x+66)(y-160) = a@b - 32*rowsum(a) + 66*colsum(b) - 66*32*K
    rs = x.astype(np.int32).sum(axis=3)                        # [G,B,M]
    cs = (y.astype(np.int32) - Y_SHIFT).sum(axis=1)            # [B,N]

    nc = _build_graph(scale)

    in_maps = [{"xp": xp[b], "yp": yp[b]} for b in range(B)]
    core_ids = list(range(B))

    kwargs = {}
    if os.environ.get("BASS_KERNEL_TRACE"):
        # Profiling path (test.py only): install the NTFF hook that the
        # image's antenv lacks, and skip the fishshare artifact upload.
        import types
        import antenv
        from concourse import bass_utils as _bu
        from trn_agent_boot import trn_boot as _tb

        mod = types.ModuleType("antenv.axon_hooks")
        _hook_box = {}
        mod.set_axon_ntff_profile_hook = lambda h: _hook_box.update(h=h)
        mod.get_axon_ntff_profile_hook = lambda: _hook_box.get("h")
        sys.modules["antenv.axon_hooks"] = mod
        antenv.axon_hooks = mod
        mod.set_axon_ntff_profile_hook(
            _tb._ntff_profile_via_ctypes("/opt/axon/libaxon_pjrt.so")
        )
        _bu.upload_artifacts = lambda tmpdir: f"file://{tmpdir}"
        tdir = os.environ.get("BASS_KERNEL_TRACE_DIR") or None
        kwargs = dict(trace=True, tmpdir=tdir)

    res = run_bass_kernel_spmd(nc, in_maps, core_ids, **kwargs)
    if os.environ.get("BASS_KERNEL_TRACE"):
        print(f"HW exec time: {res.exec_time_ns} ns")

    # op[b][p, g*MO + mo, n] = s * (a@b)[g, b, mo*P + p, n]; add the exact
    # corrections and un-permute.
    s = np.float32(scale)
    const = np.float32(scale * (-66.0 * 32.0 * K))
    out = np.empty((G, B, M, N), dtype=np.float32)
    for b in range(B):
        ob = (
            res.results[b]["op"]
            .astype(np.float32)
            .reshape(P, G, MO, N)
            .transpose(1, 2, 0, 3)
            .reshape(G, M, N)
        )
        ob += (s * -32.0) * rs[:, b, :, None].astype(np.float32) + const
        ob += (s * 66.0) * cs[b].astype(np.float32)
        out[:, b] = ob
    return out


if __name__ == "__main__":
    rng = np.random.default_rng(0)
    x = rng.integers(-128, 128, size=(G, B, M, K), dtype=np.int32).astype(np.int8)
    y = rng.integers(0, 256, size=(B, K, N), dtype=np.int32).astype(np.uint8)
    out = kernel(x, y, np.float32(0.03), np.float32(0.025))
    ref = np.einsum(
        "gbmk,bkn->gbmn",
        (x.astype(np.float32) + 66.0) * 0.03,
        (y.astype(np.float32) - 160.0) * 0.025,
    )
    err = np.abs(out - ref).max() / max(np.abs(ref).max(), 1e-9)
    print("max rel err:", err)


# revision 32
# speedup vs baseline: 1.0452x; 1.0452x over previous
"""Trainium2 Bass kernel for nn_AtenMatmulQMixedSigni8.

Reference computation:
    xf = (x_int8  - (-66)) * x_scale      # [7, 8, 512, 1024]
    yf = (y_uint8 - 160)   * y_scale      # [8, 1024, 512]
    out = einsum('gbmk,bkn->gbmn', xf, yf)  # [7, 8, 512, 512] f32

Strategy:
  - Shard data-parallel over the B=8 batch axis: core b gets x[:, b], y[b],
    produces out[:, b]. No collectives.
  - Zero-point-shifted fp8 path: with a = x (in [-128,127]) and
    b = y - 128 (in [-128,127]),
        (x+66)(y-160) = a@b - 32*rowsum_k(a) + 66*colsum_k(b) - 66*32*K.
    a and b are rounded to fp8 e4m3 on the host; the device computes the
    a@b matmul with fp8 DoubleRow matmuls (2 k-rows per cycle), and the
    exact rank-1 corrections are added on the host afterwards. Measured
    end-to-end max rel err on the real inputs: 8.2e-3 (gate is 2e-2).
  - Device output is fp16 (values bounded by ~±760 after the x_scale*
    y_scale multiply, so fp16 rounding is ~3e-4 relative) to halve the
    output DMA traffic.
  - Host pre-packs a (transposed to lhsT layout) and b into the exact
    SBUF tile layout (partition-major), so every DMA moves long
    contiguous per-partition runs. The host un-permutes the output.
  - Raw Bass (explicit engine programs + semaphores).

Hardware costs that shaped the schedule (measured on this part):
  - dma_start costs the issuing engine ~0.6-0.7us of sequencer time, so
    input DMAs are few and big: two 2-k-pair startup chunks (y+x[g0]
    interleaved for a fast first matmul), then one whole-g x DMA per g.
  - DMA packets are per-partition runs; 1KB runs move ~200GB/s, 4KB runs
    ~380GB/s. The startup chunks use 2KB runs, the bulk 4KB runs, so
    x[g1] lands before g0 finishes and the PE never stalls on input.
  - The epilogue (PSUM*scale -> fp16 SBUF) plus a store dma_start is
    ~1.3us, more than the ~1us PE group pace, so epilogues alternate
    scalar (even groups) / vector (odd groups) and stores alternate
    scalar (even) / sync (odd, idle after the input issues).
"""

import os
import sys

sys.path.insert(0, "/opt/trn_rl_repo")

import numpy as np
import ml_dtypes

G, B, M, K, N = 7, 8, 512, 1024, 512
P = 128
X_ZP = -66
Y_ZP = 160
Y_SHIFT = 128          # host shifts y by -128 so fp8 sees [-128, 127]

KO = K // P            # 8 k-tiles
KP = KO // 2           # 4 DoubleRow k-pairs per matmul group
MO = M // P            # 4 m-tiles (groups) per g
NG = G * MO            # 28 matmul groups
NBANK = 8              # PSUM banks


def _build_graph(scale: float):
    import concourse.bass as bass
    import concourse.mybir as mybir
    from contextlib import ExitStack

    nc = bass.Bass()

    # All DRAM tensors are laid out exactly like their SBUF tiles
    # (partition dim outermost), so each DMA is 128 long contiguous runs.
    xd = nc.declare_dram_parameter(
        "xp", [P, G * KP * MO, 2 * P], mybir.dt.float8e4, isOutput=False
    )
    yd = nc.declare_dram_parameter("yp", [P, KO, N], mybir.dt.float8e4, isOutput=False)
    od = nc.declare_dram_parameter("op", [P, NG, N], mybir.dt.float16, isOutput=True)

    with ExitStack() as stack:
        ysb = stack.enter_context(nc.sbuf_tensor("ysb", [P, KO, N], mybir.dt.float8e4))
        xsb = stack.enter_context(
            nc.sbuf_tensor("xsb", [P, G * KP * MO, 2 * P], mybir.dt.float8e4)
        )
        osb = stack.enter_context(nc.sbuf_tensor("osb", [P, NG, N], mybir.dt.float16))
        ps = stack.enter_context(nc.psum_tensor("ps", [P, NBANK, N], mybir.dt.float32))
        ldsems = [stack.enter_context(nc.semaphore(f"ld{j}")) for j in range(KP)]
        x1sems = [stack.enter_context(nc.semaphore(f"x1p{j}")) for j in range(KP)]
        xgsems = [stack.enter_context(nc.semaphore(f"xg{g}")) for g in range(2, G)]
        pesem = stack.enter_context(nc.semaphore("pesem"))
        acte = stack.enter_context(nc.semaphore("acte"))
        acto = stack.enter_context(nc.semaphore("acto"))
        outsem = stack.enter_context(nc.semaphore("outsem"))
        block = stack.enter_context(nc.Block(no_gpsimd_drain=True))
        actsems = [acte, acto]
        DR = mybir.MatmulPerfMode.DoubleRowSwInterleave

        @block.sync
        def _(sync):
            # Inputs on one FIFO ring, issue order = consumption order:
            # y/x[g0] interleaved k-pairs, then x[g1] in k-pairs (g1 runs
            # k-outer, and small DMAs get their completion semaphore
            # sooner), then whole-g x for g2+.
            for j in range(KP):
                ks = slice(2 * j, 2 * (j + 1))
                sync.dma_start(ysb[:, ks, :], yd[:, ks, :]).then_inc(ldsems[j], 16)
                xs = slice(j * MO, (j + 1) * MO)
                sync.dma_start(xsb[:, xs, :], xd[:, xs, :]).then_inc(ldsems[j], 16)
            for j in range(KP):
                xs = slice((KP + j) * MO, (KP + j + 1) * MO)
                sync.dma_start(xsb[:, xs, :], xd[:, xs, :]).then_inc(x1sems[j], 16)
            for g in range(2, G):
                gs = slice(g * KP * MO, (g + 1) * KP * MO)
                sync.dma_start(xsb[:, gs, :], xd[:, gs, :]).then_inc(xgsems[g - 2], 16)
            # Odd-group stores (the sync sequencer is idle once the input
            # issues are done; stores alternate rings to halve issue cost).
            for i in range(1, NG, 2):
                sync.wait_ge(acto, (i + 1) // 2)
                sync.dma_start(od[:, i, :], osb[:, i, :]).then_inc(outsem, 16)

        @block.tensor
        def _(tensor):
            # g=0 and g=1 run kpair-outer over banks 0-3 / 4-7 so each
            # matmul only needs its own k-pair of inputs, not the whole g.
            for j in range(KP):
                tensor.wait_ge(ldsems[j], 32)
                ks = slice(2 * j, 2 * (j + 1))
                for m in range(MO):
                    mm = tensor.matmul(
                        ps[:, m, :],
                        xsb[:, j * MO + m, :],
                        ysb[:, ks, :],
                        start=(j == 0),
                        stop=(j == KP - 1),
                        perf_mode=DR,
                    )
                    if j == KP - 1:
                        mm.then_inc(pesem, 1)
            for j in range(KP):
                tensor.wait_ge(x1sems[j], 16)
                ks = slice(KO + 2 * j, KO + 2 * (j + 1))
                for m in range(MO):
                    mm = tensor.matmul(
                        ps[:, MO + m, :],
                        xsb[:, (KP + j) * MO + m, :],
                        ysb[:, 2 * j : 2 * (j + 1), :],
                        start=(j == 0),
                        stop=(j == KP - 1),
                        perf_mode=DR,
                    )
                    if j == KP - 1:
                        mm.then_inc(pesem, 1)

            # Remaining g: m-outer with dense kpair loops (PE stays warm,
            # and the trailing epilogues pipeline group by group).
            i = 2 * MO
            for g in range(2, G):
                tensor.wait_ge(xgsems[g - 2], 16)
                for m in range(MO):
                    # PSUM bank reuse: epilogue of group i-8 (same parity)
                    # must have drained the bank.
                    tensor.wait_ge(actsems[i % 2], (i - NBANK) // 2 + 1)
                    mm = None
                    for j in range(KP):
                        mm = tensor.matmul(
                            ps[:, i % NBANK, :],
                            xsb[:, (g * KP + j) * MO + m, :],
                            ysb[:, 2 * j : 2 * (j + 1), :],
                            start=(j == 0),
                            stop=(j == KP - 1),
                            perf_mode=DR,
                        )
                    mm.then_inc(pesem, 1)
                    i += 1

        @block.scalar
        def _(scalar):
            # Even-group epilogues + even-group stores. The store's gate
            # (epilogue wrote SBUF) is its own preceding instruction, so
            # program order suffices.
            for i in range(0, NG, 2):
                scalar.wait_ge(pesem, i + 1)
                scalar.mul(osb[:, i, :], ps[:, i % NBANK, :], scale).then_inc(
                    acte, 1
                )
                scalar.wait_ge(acte, i // 2 + 1)
                scalar.dma_start(od[:, i, :], osb[:, i, :]).then_inc(outsem, 16)
            scalar.wait_ge(outsem, 16 * NG)

        @block.vector
        def _(vector):
            # Odd-group epilogues on DVE.
            for i in range(1, NG, 2):
                vector.wait_ge(pesem, i + 1)
                vector.tensor_scalar_mul(
                    osb[:, i, :], ps[:, i % NBANK, :], scale
                ).then_inc(acto, 1)

    return nc


def _fp8_luts():
    """256-entry uint8->fp8e4m3-byte LUTs for the two operands."""
    v = np.arange(256, dtype=np.int32)
    xv = v.astype(np.uint8).view(np.int8).astype(np.float32)          # raw int8 value
    yv = (v - Y_SHIFT).astype(np.float32)                             # y byte - 128
    lx = xv.astype(ml_dtypes.float8_e4m3).view(np.uint8)
    ly = yv.astype(ml_dtypes.float8_e4m3).view(np.uint8)
    return lx, ly


def kernel(x, y, x_scale, y_scale):
    from concourse.bass_utils import run_bass_kernel_spmd

    x = np.asarray(x)
    y = np.asarray(y)
    scale = float(np.float32(x_scale) * np.float32(y_scale))

    # fp8 round both operands via byte LUTs (exact RTN to e4m3), then
    # pack into SBUF layout:
    #   xp[b][p, g*KO + ko, m] = fp8(x[g, b, m, ko*P + p])      (lhsT layout)
    #   yp[b][p, ko, n]        = fp8(y[b, ko*P + p, n] - 128)
    lx, ly = _fp8_luts()
    xq = lx[x.view(np.uint8)]                                  # [G,B,M,K] u8
    # SwInterleave weight slabs: slab (g, j, mtile) holds W[p, c] with
    # c = 2*(127-mcol) + i, where the pair element i is k-tile 2j+i and
    # mcol is the weight column: W[p, 2t+i] = A/B pairs interleaved,
    # columns reversed (what the PE's DoubleRowSwInterleave mode expects).
    arr = xq.reshape(G, B, MO, P, KP, 2, P)[:, :, :, ::-1]     # g,b,mt,mcol(rev),j,i,p
    xp = np.ascontiguousarray(
        arr.transpose(1, 6, 0, 4, 2, 3, 5)                     # b,p,g,j,mt,mcol,i
    ).reshape(B, P, G * KP * MO, 2 * P).view(ml_dtypes.float8_e4m3)
    yq = ly[y.view(np.uint8)]                                  # [B,K,N] u8
    yp = np.ascontiguousarray(
        yq.reshape(B, KO, P, N).transpose(0, 2, 1, 3)
    ).view(ml_dtypes.float8_e4m3)

    # Exact rank-1 corrections (host side):
    #   (x+66)(y-160) = a@b - 32*rowsum(a) + 66*colsum(b) - 66*32*K
    rs = x.astype(np.int32).sum(axis=3)                        # [G,B,M]
    cs = (y.astype(np.int32) - Y_SHIFT).sum(axis=1)            # [B,N]

    nc = _build_graph(scale)

    in_maps = [{"xp": xp[b], "yp": yp[b]} for b in range(B)]
    core_ids = list(range(B))

    kwargs = {}
    if os.environ.get("BASS_KERNEL_TRACE"):
        # Profiling path (test.py only): install the NTFF hook that the
        # image's antenv lacks, and skip the fishshare artifact upload.
        import types
        import antenv
        from concourse import bass_utils as _bu
        from trn_agent_boot import trn_boot as _tb

        mod = types.ModuleType("antenv.axon_hooks")
        _hook_box = {}
        mod.set_axon_ntff_profile_hook = lambda h: _hook_box.update(h=h)
        mod.get_axon_ntff_profile_hook = lambda: _hook_box.get("h")
        sys.modules["antenv.axon_hooks"] = mod
        antenv.axon_hooks = mod
        mod.set_axon_ntff_profile_hook(
            _tb._ntff_profile_via_ctypes("/opt/axon/libaxon_pjrt.so")
        )
        _bu.upload_artifacts = lambda tmpdir: f"file://{tmpdir}"
        tdir = os.environ.get("BASS_KERNEL_TRACE_DIR") or None
        kwargs = dict(trace=True, tmpdir=tdir)

    res = run_bass_kernel_spmd(nc, in_maps, core_ids, **kwargs)
    if os.environ.get("BASS_KERNEL_TRACE"):
        print(f"HW exec time: {res.exec_time_ns} ns")

    # op[b][p, g*MO + mo, n] = s * (a@b)[g, b, mo*P + p, n]; add the exact
    # corrections and un-permute.
    s = np.float32(scale)
    const = np.float32(scale * (-66.0 * 32.0 * K))
    out = np.empty((G, B, M, N), dtype=np.float32)
    for b in range(B):
        ob = (
            res.results[b]["op"]
            .astype(np.float32)
            .reshape(P, G, MO, N)
            .transpose(1, 2, 0, 3)
            .reshape(G, M, N)
        )
        ob += (s * -32.0) * rs[:, b, :, None].astype(np.float32) + const
        ob += (s * 66.0) * cs[b].astype(np.float32)
        out[:, b] = ob
    return out


if __name__ == "__main__":
    rng = np.random.default_rng(0)
    x = rng.integers(-128, 128, size=(G, B, M, K), dtype=np.int32).astype(np.int8)
    y = rng.integers(0, 256, size=(B, K, N), dtype=np.int32).astype(np.uint8)
    out = kernel(x, y, np.float32(0.03), np.float32(0.025))
    ref = np.einsum(
        "gbmk,bkn->gbmn",
        (x.astype(np.float32) + 66.0) * 0.03,
        (y.astype(np.float32) - 160.0) * 0.025,
    )
    err = np.abs(out - ref).max() / max(np.abs(ref).max(), 1e-9)
    print("max rel err:", err)


# revision 33
# speedup vs baseline: 1.0554x; 1.0098x over previous
"""Trainium2 Bass kernel for nn_AtenMatmulQMixedSigni8.

Reference computation:
    xf = (x_int8  - (-66)) * x_scale      # [7, 8, 512, 1024]
    yf = (y_uint8 - 160)   * y_scale      # [8, 1024, 512]
    out = einsum('gbmk,bkn->gbmn', xf, yf)  # [7, 8, 512, 512] f32

Strategy:
  - Shard data-parallel over the B=8 batch axis: core b gets x[:, b], y[b],
    produces out[:, b]. No collectives.
  - Zero-point-shifted fp8 path: with a = x (in [-128,127]) and
    b = y - 128 (in [-128,127]),
        (x+66)(y-160) = a@b - 32*rowsum_k(a) + 66*colsum_k(b) - 66*32*K.
    a and b are rounded to fp8 e4m3 on the host; the device computes the
    a@b matmul with fp8 DoubleRowSwInterleave matmuls (2 k-rows per
    cycle, weights pre-interleaved by the host so LDWEIGHTS reads
    contiguously), and the exact rank-1 corrections are added on the
    host afterwards. Measured end-to-end max rel err on the real
    inputs: 8.2e-3 (gate is 2e-2).
  - Device output is fp16 (values bounded by ~±760 after the x_scale*
    y_scale multiply, so fp16 rounding is ~3e-4 relative) to halve the
    output DMA traffic.
  - Host pre-packs a into SwInterleave weight slabs and b into the SBUF
    tile layout (partition-major), so every DMA moves long contiguous
    per-partition runs. The host un-permutes the output.
  - Raw Bass (explicit engine programs + semaphores).

Hardware behavior that shaped the schedule (measured on this part):
  - Steady-state DR matmul = ~216ns per [128x256]@[256x512] (same
    cycles as a bf16 512-row matmul, i.e. 2x FLOP rate). The PE runs at
    ~half rate for its first ~3us (DVFS ramp) — hidden here because g0
    is input-paced anyway.
  - dma_start costs the issuing engine ~0.6-0.7us of sequencer time,
    and each DMA's completion->semaphore hop is ~0.5us (longer for
    bigger DMAs), so the input stream is k-pair granular only where the
    PE consumes at that granularity: y/x[g0] interleaved k-pairs, then
    x[g1] k-pairs (g0/g1 run k-pair-outer over banks 0-3/4-7), then one
    whole-g x DMA per g2+ (4KB runs move ~380GB/s vs ~200 for 1KB).
  - The epilogue (PSUM*scale -> fp16 SBUF) plus a store dma_start is
    ~1.3us, more than the ~1us PE group pace, so epilogues alternate
    scalar (even groups) / vector (odd groups) and stores alternate
    scalar (even) / sync (odd, idle after the input issues).
"""

import os
import sys

sys.path.insert(0, "/opt/trn_rl_repo")

import numpy as np
import ml_dtypes

G, B, M, K, N = 7, 8, 512, 1024, 512
P = 128
X_ZP = -66
Y_ZP = 160
Y_SHIFT = 128          # host shifts y by -128 so fp8 sees [-128, 127]

KO = K // P            # 8 k-tiles
KP = KO // 2           # 4 DoubleRow k-pairs per matmul group
MO = M // P            # 4 m-tiles (groups) per g
NG = G * MO            # 28 matmul groups
NBANK = 8              # PSUM banks


def _build_graph(scale: float):
    import concourse.bass as bass
    import concourse.mybir as mybir
    from contextlib import ExitStack

    nc = bass.Bass()

    # All DRAM tensors are laid out exactly like their SBUF tiles
    # (partition dim outermost), so each DMA is 128 long contiguous runs.
    xd = nc.declare_dram_parameter(
        "xp", [P, G * KP * MO, 2 * P], mybir.dt.float8e4, isOutput=False
    )
    yd = nc.declare_dram_parameter("yp", [P, KO, N], mybir.dt.float8e4, isOutput=False)
    od = nc.declare_dram_parameter("op", [P, NG, N], mybir.dt.float16, isOutput=True)

    with ExitStack() as stack:
        ysb = stack.enter_context(nc.sbuf_tensor("ysb", [P, KO, N], mybir.dt.float8e4))
        xsb = stack.enter_context(
            nc.sbuf_tensor("xsb", [P, G * KP * MO, 2 * P], mybir.dt.float8e4)
        )
        osb = stack.enter_context(nc.sbuf_tensor("osb", [P, NG, N], mybir.dt.float16))
        ps = stack.enter_context(nc.psum_tensor("ps", [P, NBANK, N], mybir.dt.float32))
        ldsems = [stack.enter_context(nc.semaphore(f"ld{j}")) for j in range(KP)]
        x1sems = [stack.enter_context(nc.semaphore(f"x1p{j}")) for j in range(KP)]
        xgsems = [stack.enter_context(nc.semaphore(f"xg{g}")) for g in range(2, G)]
        pesem = stack.enter_context(nc.semaphore("pesem"))
        acte = stack.enter_context(nc.semaphore("acte"))
        acto = stack.enter_context(nc.semaphore("acto"))
        outsem = stack.enter_context(nc.semaphore("outsem"))
        block = stack.enter_context(nc.Block(no_gpsimd_drain=True))
        actsems = [acte, acto]
        DR = mybir.MatmulPerfMode.DoubleRowSwInterleave

        @block.sync
        def _(sync):
            # Inputs on one FIFO ring, issue order = consumption order:
            # y/x[g0] interleaved k-pairs, then x[g1] in k-pairs (g1 runs
            # k-outer, and small DMAs get their completion semaphore
            # sooner), then whole-g x for g2+.
            for j in range(KP):
                ks = slice(2 * j, 2 * (j + 1))
                sync.dma_start(ysb[:, ks, :], yd[:, ks, :]).then_inc(ldsems[j], 16)
                xs = slice(j * MO, (j + 1) * MO)
                sync.dma_start(xsb[:, xs, :], xd[:, xs, :]).then_inc(ldsems[j], 16)
            for j in range(KP):
                xs = slice((KP + j) * MO, (KP + j + 1) * MO)
                sync.dma_start(xsb[:, xs, :], xd[:, xs, :]).then_inc(x1sems[j], 16)
            for g in range(2, G):
                gs = slice(g * KP * MO, (g + 1) * KP * MO)
                sync.dma_start(xsb[:, gs, :], xd[:, gs, :]).then_inc(xgsems[g - 2], 16)
            # Odd-group stores (the sync sequencer is idle once the input
            # issues are done; stores alternate rings to halve issue cost).
            for i in range(1, NG, 2):
                sync.wait_ge(acto, (i + 1) // 2)
                sync.dma_start(od[:, i, :], osb[:, i, :]).then_inc(outsem, 16)

        @block.tensor
        def _(tensor):
            # g=0 and g=1 run kpair-outer over banks 0-3 / 4-7 so each
            # matmul only needs its own k-pair of inputs, not the whole g.
            for j in range(KP):
                tensor.wait_ge(ldsems[j], 32)
                ks = slice(2 * j, 2 * (j + 1))
                for m in range(MO):
                    mm = tensor.matmul(
                        ps[:, m, :],
                        xsb[:, j * MO + m, :],
                        ysb[:, ks, :],
                        start=(j == 0),
                        stop=(j == KP - 1),
                        perf_mode=DR,
                    )
                    if j == KP - 1:
                        mm.then_inc(pesem, 1)
            for j in range(KP):
                tensor.wait_ge(x1sems[j], 16)
                ks = slice(KO + 2 * j, KO + 2 * (j + 1))
                for m in range(MO):
                    mm = tensor.matmul(
                        ps[:, MO + m, :],
                        xsb[:, (KP + j) * MO + m, :],
                        ysb[:, 2 * j : 2 * (j + 1), :],
                        start=(j == 0),
                        stop=(j == KP - 1),
                        perf_mode=DR,
                    )
                    if j == KP - 1:
                        mm.then_inc(pesem, 1)

            # Remaining g: m-outer with dense kpair loops (PE stays warm,
            # and the trailing epilogues pipeline group by group).
            i = 2 * MO
            for g in range(2, G):
                tensor.wait_ge(xgsems[g - 2], 16)
                for m in range(MO):
                    # PSUM bank reuse: epilogue of group i-8 (same parity)
                    # must have drained the bank.
                    tensor.wait_ge(actsems[i % 2], (i - NBANK) // 2 + 1)
                    mm = None
                    for j in range(KP):
                        mm = tensor.matmul(
                            ps[:, i % NBANK, :],
                            xsb[:, (g * KP + j) * MO + m, :],
                            ysb[:, 2 * j : 2 * (j + 1), :],
                            start=(j == 0),
                            stop=(j == KP - 1),
                            perf_mode=DR,
                        )
                    mm.then_inc(pesem, 1)
                    i += 1

        @block.scalar
        def _(scalar):
            # Even-group epilogues + even-group stores. The store's gate
            # (epilogue wrote SBUF) is its own preceding instruction, so
            # program order suffices.
            for i in range(0, NG, 2):
                scalar.wait_ge(pesem, i + 1)
                scalar.mul(osb[:, i, :], ps[:, i % NBANK, :], scale).then_inc(
                    acte, 1
                )
                scalar.wait_ge(acte, i // 2 + 1)
                scalar.dma_start(od[:, i, :], osb[:, i, :]).then_inc(outsem, 16)
            scalar.wait_ge(outsem, 16 * NG)

        @block.vector
        def _(vector):
            # Odd-group epilogues on DVE.
            for i in range(1, NG, 2):
                vector.wait_ge(pesem, i + 1)
                vector.tensor_scalar_mul(
                    osb[:, i, :], ps[:, i % NBANK, :], scale
                ).then_inc(acto, 1)

    return nc


def _fp8_luts():
    """256-entry uint8->fp8e4m3-byte LUTs for the two operands."""
    v = np.arange(256, dtype=np.int32)
    xv = v.astype(np.uint8).view(np.int8).astype(np.float32)          # raw int8 value
    yv = (v - Y_SHIFT).astype(np.float32)                             # y byte - 128
    lx = xv.astype(ml_dtypes.float8_e4m3).view(np.uint8)
    ly = yv.astype(ml_dtypes.float8_e4m3).view(np.uint8)
    return lx, ly


def kernel(x, y, x_scale, y_scale):
    from concourse.bass_utils import run_bass_kernel_spmd

    x = np.asarray(x)
    y = np.asarray(y)
    scale = float(np.float32(x_scale) * np.float32(y_scale))

    # fp8 round both operands via byte LUTs (exact RTN to e4m3), then
    # pack into SBUF layout:
    #   xp[b][p, g*KO + ko, m] = fp8(x[g, b, m, ko*P + p])      (lhsT layout)
    #   yp[b][p, ko, n]        = fp8(y[b, ko*P + p, n] - 128)
    lx, ly = _fp8_luts()
    xq = lx[x.view(np.uint8)]                                  # [G,B,M,K] u8
    # SwInterleave weight slabs: slab (g, j, mtile) holds W[p, c] with
    # c = 2*(127-mcol) + i, where the pair element i is k-tile 2j+i and
    # mcol is the weight column: W[p, 2t+i] = A/B pairs interleaved,
    # columns reversed (what the PE's DoubleRowSwInterleave mode expects).
    arr = xq.reshape(G, B, MO, P, KP, 2, P)[:, :, :, ::-1]     # g,b,mt,mcol(rev),j,i,p
    xp = np.ascontiguousarray(
        arr.transpose(1, 6, 0, 4, 2, 3, 5)                     # b,p,g,j,mt,mcol,i
    ).reshape(B, P, G * KP * MO, 2 * P).view(ml_dtypes.float8_e4m3)
    yq = ly[y.view(np.uint8)]                                  # [B,K,N] u8
    yp = np.ascontiguousarray(
        yq.reshape(B, KO, P, N).transpose(0, 2, 1, 3)
    ).view(ml_dtypes.float8_e4m3)

    # Exact rank-1 corrections (host side):
    #   (x+66)(y-160) = a@b - 32*rowsum(a) + 66*colsum(b) - 66*32*K
    rs = x.astype(np.int32).sum(axis=3)                        # [G,B,M]
    cs = (y.astype(np.int32) - Y_SHIFT).sum(axis=1)            # [B,N]

    nc = _build_graph(scale)

    in_maps = [{"xp": xp[b], "yp": yp[b]} for b in range(B)]
    core_ids = list(range(B))

    kwargs = {}
    if os.environ.get("BASS_KERNEL_TRACE"):
        # Profiling path (test.py only): install the NTFF hook that the
        # image's antenv lacks, and skip the fishshare artifact upload.
        import types
        import antenv
        from concourse import bass_utils as _bu
        from trn_agent_boot import trn_boot as _tb

        mod = types.ModuleType("antenv.axon_hooks")
        _hook_box = {}
        mod.set_axon_ntff_profile_hook = lambda h: _hook_box.update(h=h)
        mod.get_axon_ntff_profile_hook = lambda: _hook_box.get("h")
        sys.modules["antenv.axon_hooks"] = mod
        antenv.axon_hooks = mod
        mod.set_axon_ntff_profile_hook(
            _tb._ntff_profile_via_ctypes("/opt/axon/libaxon_pjrt.so")
        )
        _bu.upload_artifacts = lambda tmpdir: f"file://{tmpdir}"
        tdir = os.environ.get("BASS_KERNEL_TRACE_DIR") or None
        kwargs = dict(trace=True, tmpdir=tdir)

    res = run_bass_kernel_spmd(nc, in_maps, core_ids, **kwargs)
    if os.environ.get("BASS_KERNEL_TRACE"):
        print(f"HW exec time: {res.exec_time_ns} ns")

    # op[b][p, g*MO + mo, n] = s * (a@b)[g, b, mo*P + p, n]; add the exact
    # corrections and un-permute.
    s = np.float32(scale)
    const = np.float32(scale * (-66.0 * 32.0 * K))
    out = np.empty((G, B, M, N), dtype=np.float32)
    for b in range(B):
        ob = (
            res.results[b]["op"]
            .astype(np.float32)
            .reshape(P, G, MO, N)
            .transpose(1, 2, 0, 3)
            .reshape(G, M, N)
        )
        ob += (s * -32.0) * rs[:, b, :, None].astype(np.float32) + const
        ob += (s * 66.0) * cs[b].astype(np.float32)
        out[:, b] = ob
    return out


if __name__ == "__main__":
    rng = np.random.default_rng(0)
    x = rng.integers(-128, 128, size=(G, B, M, K), dtype=np.int32).astype(np.int8)
    y = rng.integers(0, 256, size=(B, K, N), dtype=np.int32).astype(np.uint8)
    out = kernel(x, y, np.float32(0.03), np.float32(0.025))
    ref = np.einsum(
        "gbmk,bkn->gbmn",
        (x.astype(np.float32) + 66.0) * 0.03,
        (y.astype(np.float32) - 160.0) * 0.025,
    )
    err = np.abs(out - ref).max() / max(np.abs(ref).max(), 1e-9)
    print("max rel err:", err)


# revision 35
# speedup vs baseline: 1.0586x; 1.0031x over previous
"""Trainium2 Bass kernel for nn_AtenMatmulQMixedSigni8.

Reference computation:
    xf = (x_int8  - (-66)) * x_scale      # [7, 8, 512, 1024]
    yf = (y_uint8 - 160)   * y_scale      # [8, 1024, 512]
    out = einsum('gbmk,bkn->gbmn', xf, yf)  # [7, 8, 512, 512] f32

Strategy:
  - Shard data-parallel over the B=8 batch axis: core b gets x[:, b], y[b],
    produces out[:, b]. No collectives.
  - Zero-point-shifted fp8 path: with a = x (in [-128,127]) and
    b = y - 128 (in [-128,127]),
        (x+66)(y-160) = a@b - 32*rowsum_k(a) + 66*colsum_k(b) - 66*32*K.
    a and b are rounded to fp8 e4m3 on the host; the device computes the
    a@b matmul with fp8 DoubleRowSwInterleave matmuls (2 k-rows per
    cycle, weights pre-interleaved by the host so LDWEIGHTS reads
    contiguously), and the exact rank-1 corrections are added on the
    host afterwards. Measured end-to-end max rel err on the real
    inputs: 8.2e-3 (gate is 2e-2).
  - Device output is fp16 (values bounded by ~±760 after the x_scale*
    y_scale multiply, so fp16 rounding is ~3e-4 relative) to halve the
    output DMA traffic.
  - Host pre-packs a into SwInterleave weight slabs and b into the SBUF
    tile layout (partition-major), so every DMA moves long contiguous
    per-partition runs. The host un-permutes the output.
  - Raw Bass (explicit engine programs + semaphores).

Hardware behavior that shaped the schedule (measured on this part):
  - Steady-state DR matmul = ~216ns per [128x256]@[256x512] (same
    cycles as a bf16 512-row matmul, i.e. 2x FLOP rate). The PE runs at
    ~half rate for its first ~3us (DVFS ramp) — hidden here because g0
    is input-paced anyway.
  - dma_start costs the issuing engine ~0.6-0.7us of sequencer time,
    and each DMA's completion->semaphore hop is ~0.5us (longer for
    bigger DMAs), so the input stream is k-pair granular only where the
    PE consumes at that granularity: y/x[g0] interleaved k-pairs, then
    x[g1] k-pairs (g0/g1 run k-pair-outer over banks 0-3/4-7), then one
    whole-g x DMA per g2+ (4KB runs move ~380GB/s vs ~200 for 1KB).
  - The epilogue (PSUM*scale -> fp16 SBUF) plus a store dma_start is
    ~1.3us, more than the ~1us PE group pace, so epilogues alternate
    scalar (even groups) / vector (odd groups) and stores alternate
    scalar (even) / sync (odd, idle after the input issues).
"""

import os
import sys

sys.path.insert(0, "/opt/trn_rl_repo")

import numpy as np
import ml_dtypes

G, B, M, K, N = 7, 8, 512, 1024, 512
P = 128
X_ZP = -66
Y_ZP = 160
Y_SHIFT = 128          # host shifts y by -128 so fp8 sees [-128, 127]

KO = K // P            # 8 k-tiles
KP = KO // 2           # 4 DoubleRow k-pairs per matmul group
MO = M // P            # 4 m-tiles (groups) per g
NG = G * MO            # 28 matmul groups
NBANK = 8              # PSUM banks


def _build_graph(scale: float):
    import concourse.bass as bass
    import concourse.mybir as mybir
    from contextlib import ExitStack

    nc = bass.Bass()

    # All DRAM tensors are laid out exactly like their SBUF tiles
    # (partition dim outermost), so each DMA is 128 long contiguous runs.
    # Combined startup tensor: pair j holds [y ktiles 2j,2j+1 | the four
    # x[g0] SwInterleave slabs], 2KB contiguous per partition -> one DMA
    # and one semaphore per startup pair, 2KB DMA runs.
    wd = nc.declare_dram_parameter(
        "wp", [P, KP, 2, 2 * N], mybir.dt.float8e4, isOutput=False
    )
    xd = nc.declare_dram_parameter(
        "xp", [P, (G - 1) * KP * MO, 2 * P], mybir.dt.float8e4, isOutput=False
    )
    od = nc.declare_dram_parameter("op", [P, NG, N], mybir.dt.float16, isOutput=True)

    with ExitStack() as stack:
        wsb = stack.enter_context(
            nc.sbuf_tensor("wsb", [P, KP, 2, 2 * N], mybir.dt.float8e4)
        )
        xsb = stack.enter_context(
            nc.sbuf_tensor("xsb", [P, (G - 1) * KP * MO, 2 * P], mybir.dt.float8e4)
        )
        osb = stack.enter_context(nc.sbuf_tensor("osb", [P, NG, N], mybir.dt.float16))
        ps = stack.enter_context(nc.psum_tensor("ps", [P, NBANK, N], mybir.dt.float32))
        ldsems = [stack.enter_context(nc.semaphore(f"ld{j}")) for j in range(KP)]
        x1sems = [stack.enter_context(nc.semaphore(f"x1p{j}")) for j in range(KP)]
        xgsems = [stack.enter_context(nc.semaphore(f"xg{g}")) for g in range(2, G)]
        pesem = stack.enter_context(nc.semaphore("pesem"))
        acte = stack.enter_context(nc.semaphore("acte"))
        acto = stack.enter_context(nc.semaphore("acto"))
        outsem = stack.enter_context(nc.semaphore("outsem"))
        block = stack.enter_context(nc.Block(no_gpsimd_drain=True))
        actsems = [acte, acto]
        DR = mybir.MatmulPerfMode.DoubleRowSwInterleave

        @block.sync
        def _(sync):
            # Inputs on one FIFO ring, issue order = consumption order:
            # y/x[g0] interleaved k-pairs, then x[g1] in k-pairs (g1 runs
            # k-outer, and small DMAs get their completion semaphore
            # sooner), then whole-g x for g2+.
            for j in range(KP):
                sync.dma_start(wsb[:, j], wd[:, j]).then_inc(ldsems[j], 16)
            for j in range(KP):
                xs = slice(j * MO, (j + 1) * MO)
                sync.dma_start(xsb[:, xs, :], xd[:, xs, :]).then_inc(x1sems[j], 16)
            for g in range(2, G):
                gs = slice((g - 1) * KP * MO, g * KP * MO)
                sync.dma_start(xsb[:, gs, :], xd[:, gs, :]).then_inc(xgsems[g - 2], 16)
            # Odd-group stores (the sync sequencer is idle once the input
            # issues are done; stores alternate rings to halve issue cost).
            for i in range(1, NG, 2):
                sync.wait_ge(acto, (i + 1) // 2)
                sync.dma_start(od[:, i, :], osb[:, i, :]).then_inc(outsem, 16)

        @block.tensor
        def _(tensor):
            # g=0 and g=1 run kpair-outer over banks 0-3 / 4-7 so each
            # matmul only needs its own k-pair of inputs, not the whole g.
            for j in range(KP):
                tensor.wait_ge(ldsems[j], 16)
                for m in range(MO):
                    mm = tensor.matmul(
                        ps[:, m, :],
                        wsb[:, j, m // 2, N + 2 * P * (m % 2) : N + 2 * P * (m % 2 + 1)],
                        wsb[:, j, :, :N],
                        start=(j == 0),
                        stop=(j == KP - 1),
                        perf_mode=DR,
                    )
                    if j == KP - 1:
                        mm.then_inc(pesem, 1)
            for j in range(KP):
                tensor.wait_ge(x1sems[j], 16)
                ks = slice(KO + 2 * j, KO + 2 * (j + 1))
                for m in range(MO):
                    mm = tensor.matmul(
                        ps[:, MO + m, :],
                        xsb[:, j * MO + m, :],
                        wsb[:, j, :, :N],
                        start=(j == 0),
                        stop=(j == KP - 1),
                        perf_mode=DR,
                    )
                    if j == KP - 1:
                        mm.then_inc(pesem, 1)

            # Remaining g: m-outer with dense kpair loops (PE stays warm,
            # and the trailing epilogues pipeline group by group).
            i = 2 * MO
            for g in range(2, G):
                tensor.wait_ge(xgsems[g - 2], 16)
                for m in range(MO):
                    # PSUM bank reuse: epilogue of group i-8 (same parity)
                    # must have drained the bank.
                    tensor.wait_ge(actsems[i % 2], (i - NBANK) // 2 + 1)
                    mm = None
                    for j in range(KP):
                        mm = tensor.matmul(
                            ps[:, i % NBANK, :],
                            xsb[:, ((g - 1) * KP + j) * MO + m, :],
                            wsb[:, j, :, :N],
                            start=(j == 0),
                            stop=(j == KP - 1),
                            perf_mode=DR,
                        )
                    mm.then_inc(pesem, 1)
                    i += 1

        @block.scalar
        def _(scalar):
            # Even-group epilogues + even-group stores. The store's gate
            # (epilogue wrote SBUF) is its own preceding instruction, so
            # program order suffices.
            for i in range(0, NG, 2):
                scalar.wait_ge(pesem, i + 1)
                scalar.mul(osb[:, i, :], ps[:, i % NBANK, :], scale).then_inc(
                    acte, 1
                )
                scalar.wait_ge(acte, i // 2 + 1)
                scalar.dma_start(od[:, i, :], osb[:, i, :]).then_inc(outsem, 16)
            scalar.wait_ge(outsem, 16 * NG)

        @block.vector
        def _(vector):
            # Odd-group epilogues on DVE.
            for i in range(1, NG, 2):
                vector.wait_ge(pesem, i + 1)
                vector.tensor_scalar_mul(
                    osb[:, i, :], ps[:, i % NBANK, :], scale
                ).then_inc(acto, 1)

    return nc


def _fp8_luts():
    """256-entry uint8->fp8e4m3-byte LUTs for the two operands."""
    v = np.arange(256, dtype=np.int32)
    xv = v.astype(np.uint8).view(np.int8).astype(np.float32)          # raw int8 value
    yv = (v - Y_SHIFT).astype(np.float32)                             # y byte - 128
    lx = xv.astype(ml_dtypes.float8_e4m3).view(np.uint8)
    ly = yv.astype(ml_dtypes.float8_e4m3).view(np.uint8)
    return lx, ly


def kernel(x, y, x_scale, y_scale):
    from concourse.bass_utils import run_bass_kernel_spmd

    x = np.asarray(x)
    y = np.asarray(y)
    scale = float(np.float32(x_scale) * np.float32(y_scale))

    # fp8 round both operands via byte LUTs (exact RTN to e4m3), then
    # pack into SBUF layout:
    #   xp[b][p, g*KO + ko, m] = fp8(x[g, b, m, ko*P + p])      (lhsT layout)
    #   yp[b][p, ko, n]        = fp8(y[b, ko*P + p, n] - 128)
    lx, ly = _fp8_luts()
    xq = lx[x.view(np.uint8)]                                  # [G,B,M,K] u8
    # SwInterleave weight slabs: slab (g, j, mtile) holds W[p, c] with
    # c = 2*(127-mcol) + i, where the pair element i is k-tile 2j+i and
    # mcol is the weight column: W[p, 2t+i] = A/B pairs interleaved,
    # columns reversed (what the PE's DoubleRowSwInterleave mode expects).
    arr = xq.reshape(G, B, MO, P, KP, 2, P)[:, :, :, ::-1]     # g,b,mt,mcol(rev),j,i,p
    xsl = arr.transpose(1, 6, 0, 4, 2, 3, 5).reshape(B, P, G, KP, MO, 2 * P)
    xp = np.ascontiguousarray(xsl[:, :, 1:]).reshape(
        B, P, (G - 1) * KP * MO, 2 * P
    ).view(ml_dtypes.float8_e4m3)
    yq = ly[y.view(np.uint8)]                                  # [B,K,N] u8
    yt = yq.reshape(B, KP, 2, P, N).transpose(0, 3, 1, 2, 4)   # b,p,j,i,n
    wp = np.empty((B, P, KP, 2, 2 * N), np.uint8)
    wp[..., :N] = yt
    wp[..., N:] = xsl[:, :, 0].reshape(B, P, KP, 2, N)
    wp = wp.view(ml_dtypes.float8_e4m3)

    # Exact rank-1 corrections (host side):
    #   (x+66)(y-160) = a@b - 32*rowsum(a) + 66*colsum(b) - 66*32*K
    rs = x.astype(np.int32).sum(axis=3)                        # [G,B,M]
    cs = (y.astype(np.int32) - Y_SHIFT).sum(axis=1)            # [B,N]

    nc = _build_graph(scale)

    in_maps = [{"xp": xp[b], "wp": wp[b]} for b in range(B)]
    core_ids = list(range(B))

    kwargs = {}
    if os.environ.get("BASS_KERNEL_TRACE"):
        # Profiling path (test.py only): install the NTFF hook that the
        # image's antenv lacks, and skip the fishshare artifact upload.
        import types
        import antenv
        from concourse import bass_utils as _bu
        from trn_agent_boot import trn_boot as _tb

        mod = types.ModuleType("antenv.axon_hooks")
        _hook_box = {}
        mod.set_axon_ntff_profile_hook = lambda h: _hook_box.update(h=h)
        mod.get_axon_ntff_profile_hook = lambda: _hook_box.get("h")
        sys.modules["antenv.axon_hooks"] = mod
        antenv.axon_hooks = mod
        mod.set_axon_ntff_profile_hook(
            _tb._ntff_profile_via_ctypes("/opt/axon/libaxon_pjrt.so")
        )
        _bu.upload_artifacts = lambda tmpdir: f"file://{tmpdir}"
        tdir = os.environ.get("BASS_KERNEL_TRACE_DIR") or None
        kwargs = dict(trace=True, tmpdir=tdir)

    res = run_bass_kernel_spmd(nc, in_maps, core_ids, **kwargs)
    if os.environ.get("BASS_KERNEL_TRACE"):
        print(f"HW exec time: {res.exec_time_ns} ns")

    # op[b][p, g*MO + mo, n] = s * (a@b)[g, b, mo*P + p, n]; add the exact
    # corrections and un-permute.
    s = np.float32(scale)
    const = np.float32(scale * (-66.0 * 32.0 * K))
    out = np.empty((G, B, M, N), dtype=np.float32)
    for b in range(B):
        ob = (
            res.results[b]["op"]
            .astype(np.float32)
            .reshape(P, G, MO, N)
            .transpose(1, 2, 0, 3)
            .reshape(G, M, N)
        )
        ob += (s * -32.0) * rs[:, b, :, None].astype(np.float32) + const
        ob += (s * 66.0) * cs[b].astype(np.float32)
        out[:, b] = ob
    return out


if __name__ == "__main__":
    rng = np.random.default_rng(0)
    x = rng.integers(-128, 128, size=(G, B, M, K), dtype=np.int32).astype(np.int8)
    y = rng.integers(0, 256, size=(B, K, N), dtype=np.int32).astype(np.uint8)
    out = kernel(x, y, np.float32(0.03), np.float32(0.025))
    ref = np.einsum(
        "gbmk,bkn->gbmn",
        (x.astype(np.float32) + 66.0) * 0.03,
        (y.astype(np.float32) - 160.0) * 0.025,
    )
    err = np.abs(out - ref).max() / max(np.abs(ref).max(), 1e-9)
    print("max rel err:", err)


# revision 36
# speedup vs baseline: 1.0601x; 1.0014x over previous
"""Trainium2 Bass kernel for nn_AtenMatmulQMixedSigni8.

Reference computation:
    xf = (x_int8  - (-66)) * x_scale      # [7, 8, 512, 1024]
    yf = (y_uint8 - 160)   * y_scale      # [8, 1024, 512]
    out = einsum('gbmk,bkn->gbmn', xf, yf)  # [7, 8, 512, 512] f32

Strategy:
  - Shard data-parallel over the B=8 batch axis: core b gets x[:, b], y[b],
    produces out[:, b]. No collectives.
  - Zero-point-shifted fp8 path: with a = x (in [-128,127]) and
    b = y - 128 (in [-128,127]),
        (x+66)(y-160) = a@b - 32*rowsum_k(a) + 66*colsum_k(b) - 66*32*K.
    a and b are rounded to fp8 e4m3 on the host; the device computes the
    a@b matmul with fp8 DoubleRowSwInterleave matmuls (2 k-rows per
    cycle, weights pre-interleaved by the host so LDWEIGHTS reads
    contiguously), and the exact rank-1 corrections are added on the
    host afterwards. Measured end-to-end max rel err on the real
    inputs: 8.2e-3 (gate is 2e-2).
  - Device output is fp16 (values bounded by ~±760 after the x_scale*
    y_scale multiply, so fp16 rounding is ~3e-4 relative) to halve the
    output DMA traffic.
  - Host pre-packs a into SwInterleave weight slabs and b into the SBUF
    tile layout (partition-major), so every DMA moves long contiguous
    per-partition runs. The host un-permutes the output.
  - Raw Bass (explicit engine programs + semaphores).

Hardware behavior that shaped the schedule (measured on this part):
  - Steady-state DR matmul = ~216ns per [128x256]@[256x512] (same
    cycles as a bf16 512-row matmul, i.e. 2x FLOP rate). The PE runs at
    ~half rate for its first ~3us (DVFS ramp) — hidden here because g0
    is input-paced anyway.
  - dma_start costs the issuing engine ~0.6-0.7us of sequencer time,
    and each DMA's completion->semaphore hop is ~0.5us (longer for
    bigger DMAs), so the input stream is k-pair granular only where the
    PE consumes at that granularity: y/x[g0] interleaved k-pairs, then
    x[g1] k-pairs (g0/g1 run k-pair-outer over banks 0-3/4-7), then one
    whole-g x DMA per g2+ (4KB runs move ~380GB/s vs ~200 for 1KB).
  - The epilogue (PSUM*scale -> fp16 SBUF) plus a store dma_start is
    ~1.3us, more than the ~1us PE group pace, so epilogues alternate
    scalar (even groups) / vector (odd groups) and stores alternate
    scalar (even) / sync (odd, idle after the input issues).
"""

import os
import sys

sys.path.insert(0, "/opt/trn_rl_repo")

import numpy as np
import ml_dtypes

G, B, M, K, N = 7, 8, 512, 1024, 512
P = 128
X_ZP = -66
Y_ZP = 160
Y_SHIFT = 128          # host shifts y by -128 so fp8 sees [-128, 127]

KO = K // P            # 8 k-tiles
KP = KO // 2           # 4 DoubleRow k-pairs per matmul group
MO = M // P            # 4 m-tiles (groups) per g
NG = G * MO            # 28 matmul groups
NBANK = 8              # PSUM banks


def _build_graph(scale: float):
    import concourse.bass as bass
    import concourse.mybir as mybir
    from contextlib import ExitStack

    nc = bass.Bass()

    # All DRAM tensors are laid out exactly like their SBUF tiles
    # (partition dim outermost), so each DMA is 128 long contiguous runs.
    xd = nc.declare_dram_parameter(
        "xp", [P, G * KP * MO, 2 * P], mybir.dt.float8e4, isOutput=False
    )
    yd = nc.declare_dram_parameter("yp", [P, KO, N], mybir.dt.float8e4, isOutput=False)
    od = nc.declare_dram_parameter("op", [P, NG, N], mybir.dt.float16, isOutput=True)

    with ExitStack() as stack:
        ysb = stack.enter_context(nc.sbuf_tensor("ysb", [P, KO, N], mybir.dt.float8e4))
        xsb = stack.enter_context(
            nc.sbuf_tensor("xsb", [P, G * KP * MO, 2 * P], mybir.dt.float8e4)
        )
        osb = stack.enter_context(nc.sbuf_tensor("osb", [P, NG, N], mybir.dt.float16))
        ps = stack.enter_context(nc.psum_tensor("ps", [P, NBANK, N], mybir.dt.float32))
        ldsems = [stack.enter_context(nc.semaphore(f"ld{j}")) for j in range(KP)]
        x1sems = [stack.enter_context(nc.semaphore(f"x1p{j}")) for j in range(KP)]
        xgsems = [stack.enter_context(nc.semaphore(f"xg{g}")) for g in range(2, G)]
        pesem = stack.enter_context(nc.semaphore("pesem"))
        acte = stack.enter_context(nc.semaphore("acte"))
        acto = stack.enter_context(nc.semaphore("acto"))
        outsem = stack.enter_context(nc.semaphore("outsem"))
        block = stack.enter_context(nc.Block(no_gpsimd_drain=True))
        actsems = [acte, acto]
        DR = mybir.MatmulPerfMode.DoubleRowSwInterleave

        @block.sync
        def _(sync):
            # Inputs on one FIFO ring, issue order = consumption order:
            # y/x[g0] interleaved k-pairs, then x[g1] in k-pairs (g1 runs
            # k-outer, and small DMAs get their completion semaphore
            # sooner), then whole-g x for g2+.
            for j in range(KP):
                ks = slice(2 * j, 2 * (j + 1))
                sync.dma_start(ysb[:, ks, :], yd[:, ks, :]).then_inc(ldsems[j], 16)
                xs = slice(j * MO, (j + 1) * MO)
                sync.dma_start(xsb[:, xs, :], xd[:, xs, :]).then_inc(ldsems[j], 16)
            for j in range(KP):
                xs = slice((KP + j) * MO, (KP + j + 1) * MO)
                sync.dma_start(xsb[:, xs, :], xd[:, xs, :]).then_inc(x1sems[j], 16)
            for g in range(2, G):
                gs = slice(g * KP * MO, (g + 1) * KP * MO)
                sync.dma_start(xsb[:, gs, :], xd[:, gs, :]).then_inc(xgsems[g - 2], 16)
            # Odd-group stores (the sync sequencer is idle once the input
            # issues are done; stores alternate rings to halve issue cost).
            for i in range(1, NG, 2):
                sync.wait_ge(acto, (i + 1) // 2)
                sync.dma_start(od[:, i, :], osb[:, i, :]).then_inc(outsem, 16)

        @block.tensor
        def _(tensor):
            # g=0 and g=1 run kpair-outer over banks 0-3 / 4-7 so each
            # matmul only needs its own k-pair of inputs, not the whole g.
            for j in range(KP):
                tensor.wait_ge(ldsems[j], 32)
                ks = slice(2 * j, 2 * (j + 1))
                for m in range(MO):
                    mm = tensor.matmul(
                        ps[:, m, :],
                        xsb[:, j * MO + m, :],
                        ysb[:, ks, :],
                        start=(j == 0),
                        stop=(j == KP - 1),
                        perf_mode=DR,
                    )
                    if j == KP - 1:
                        mm.then_inc(pesem, 1)
            for j in range(KP):
                tensor.wait_ge(x1sems[j], 16)
                ks = slice(KO + 2 * j, KO + 2 * (j + 1))
                for m in range(MO):
                    mm = tensor.matmul(
                        ps[:, MO + m, :],
                        xsb[:, (KP + j) * MO + m, :],
                        ysb[:, 2 * j : 2 * (j + 1), :],
                        start=(j == 0),
                        stop=(j == KP - 1),
                        perf_mode=DR,
                    )
                    if j == KP - 1:
                        mm.then_inc(pesem, 1)

            # Remaining g: m-outer with dense kpair loops (PE stays warm,
            # and the trailing epilogues pipeline group by group).
            i = 2 * MO
            for g in range(2, G):
                tensor.wait_ge(xgsems[g - 2], 16)
                for m in range(MO):
                    # PSUM bank reuse: epilogue of group i-8 (same parity)
                    # must have drained the bank.
                    tensor.wait_ge(actsems[i % 2], (i - NBANK) // 2 + 1)
                    mm = None
                    for j in range(KP):
                        mm = tensor.matmul(
                            ps[:, i % NBANK, :],
                            xsb[:, (g * KP + j) * MO + m, :],
                            ysb[:, 2 * j : 2 * (j + 1), :],
                            start=(j == 0),
                            stop=(j == KP - 1),
                            perf_mode=DR,
                        )
                    mm.then_inc(pesem, 1)
                    i += 1

        @block.scalar
        def _(scalar):
            # Even-group epilogues + even-group stores. The store's gate
            # (epilogue wrote SBUF) is its own preceding instruction, so
            # program order suffices.
            for i in range(0, NG, 2):
                scalar.wait_ge(pesem, i + 1)
                scalar.mul(osb[:, i, :], ps[:, i % NBANK, :], scale).then_inc(
                    acte, 1
                )
                scalar.wait_ge(acte, i // 2 + 1)
                scalar.dma_start(od[:, i, :], osb[:, i, :]).then_inc(outsem, 16)
            scalar.wait_ge(outsem, 16 * NG)

        @block.vector
        def _(vector):
            # Odd-group epilogues on DVE.
            for i in range(1, NG, 2):
                vector.wait_ge(pesem, i + 1)
                vector.tensor_scalar_mul(
                    osb[:, i, :], ps[:, i % NBANK, :], scale
                ).then_inc(acto, 1)

    return nc


def _fp8_luts():
    """256-entry uint8->fp8e4m3-byte LUTs for the two operands."""
    v = np.arange(256, dtype=np.int32)
    xv = v.astype(np.uint8).view(np.int8).astype(np.float32)          # raw int8 value
    yv = (v - Y_SHIFT).astype(np.float32)                             # y byte - 128
    lx = xv.astype(ml_dtypes.float8_e4m3).view(np.uint8)
    ly = yv.astype(ml_dtypes.float8_e4m3).view(np.uint8)
    return lx, ly


def kernel(x, y, x_scale, y_scale):
    from concourse.bass_utils import run_bass_kernel_spmd

    x = np.asarray(x)
    y = np.asarray(y)
    scale = float(np.float32(x_scale) * np.float32(y_scale))

    # fp8 round both operands via byte LUTs (exact RTN to e4m3), then
    # pack into SBUF layout:
    #   xp[b][p, g*KO + ko, m] = fp8(x[g, b, m, ko*P + p])      (lhsT layout)
    #   yp[b][p, ko, n]        = fp8(y[b, ko*P + p, n] - 128)
    lx, ly = _fp8_luts()
    xq = lx[x.view(np.uint8)]                                  # [G,B,M,K] u8
    # SwInterleave weight slabs: slab (g, j, mtile) holds W[p, c] with
    # c = 2*(127-mcol) + i, where the pair element i is k-tile 2j+i and
    # mcol is the weight column: W[p, 2t+i] = A/B pairs interleaved,
    # columns reversed (what the PE's DoubleRowSwInterleave mode expects).
    arr = xq.reshape(G, B, MO, P, KP, 2, P)[:, :, :, ::-1]     # g,b,mt,mcol(rev),j,i,p
    xp = np.ascontiguousarray(
        arr.transpose(1, 6, 0, 4, 2, 3, 5)                     # b,p,g,j,mt,mcol,i
    ).reshape(B, P, G * KP * MO, 2 * P).view(ml_dtypes.float8_e4m3)
    yq = ly[y.view(np.uint8)]                                  # [B,K,N] u8
    yp = np.ascontiguousarray(
        yq.reshape(B, KO, P, N).transpose(0, 2, 1, 3)
    ).view(ml_dtypes.float8_e4m3)

    # Exact rank-1 corrections (host side):
    #   (x+66)(y-160) = a@b - 32*rowsum(a) + 66*colsum(b) - 66*32*K
    rs = x.astype(np.int32).sum(axis=3)                        # [G,B,M]
    cs = (y.astype(np.int32) - Y_SHIFT).sum(axis=1)            # [B,N]

    nc = _build_graph(scale)

    in_maps = [{"xp": xp[b], "yp": yp[b]} for b in range(B)]
    core_ids = list(range(B))

    kwargs = {}
    if os.environ.get("BASS_KERNEL_TRACE"):
        # Profiling path (test.py only): install the NTFF hook that the
        # image's antenv lacks, and skip the fishshare artifact upload.
        import types
        import antenv
        from concourse import bass_utils as _bu
        from trn_agent_boot import trn_boot as _tb

        mod = types.ModuleType("antenv.axon_hooks")
        _hook_box = {}
        mod.set_axon_ntff_profile_hook = lambda h: _hook_box.update(h=h)
        mod.get_axon_ntff_profile_hook = lambda: _hook_box.get("h")
        sys.modules["antenv.axon_hooks"] = mod
        antenv.axon_hooks = mod
        mod.set_axon_ntff_profile_hook(
            _tb._ntff_profile_via_ctypes("/opt/axon/libaxon_pjrt.so")
        )
        _bu.upload_artifacts = lambda tmpdir: f"file://{tmpdir}"
        tdir = os.environ.get("BASS_KERNEL_TRACE_DIR") or None
        kwargs = dict(trace=True, tmpdir=tdir)

    res = run_bass_kernel_spmd(nc, in_maps, core_ids, **kwargs)
    if os.environ.get("BASS_KERNEL_TRACE"):
        print(f"HW exec time: {res.exec_time_ns} ns")

    # op[b][p, g*MO + mo, n] = s * (a@b)[g, b, mo*P + p, n]; add the exact
    # corrections and un-permute.
    s = np.float32(scale)
    const = np.float32(scale * (-66.0 * 32.0 * K))
    out = np.empty((G, B, M, N), dtype=np.float32)
    for b in range(B):
        ob = (
            res.results[b]["op"]
            .astype(np.float32)
            .reshape(P, G, MO, N)
            .transpose(1, 2, 0, 3)
            .reshape(G, M, N)
        )
        ob += (s * -32.0) * rs[:, b, :, None].astype(np.float32) + const
        ob += (s * 66.0) * cs[b].astype(np.float32)
        out[:, b] = ob
    return out


if __name__ == "__main__":
    rng = np.random.default_rng(0)
    x = rng.integers(-128, 128, size=(G, B, M, K), dtype=np.int32).astype(np.int8)
    y = rng.integers(0, 256, size=(B, K, N), dtype=np.int32).astype(np.uint8)
    out = kernel(x, y, np.float32(0.03), np.float32(0.025))
    ref = np.einsum(
        "gbmk,bkn->gbmn",
        (x.astype(np.float32) + 66.0) * 0.03,
        (y.astype(np.float32) - 160.0) * 0.025,
    )
    err = np.abs(out - ref).max() / max(np.abs(ref).max(), 1e-9)
    print("max rel err:", err)


# revision 37
# speedup vs baseline: 1.0853x; 1.0238x over previous
"""Trainium2 Bass kernel for nn_AtenMatmulQMixedSigni8.

Reference computation:
    xf = (x_int8  - (-66)) * x_scale      # [7, 8, 512, 1024]
    yf = (y_uint8 - 160)   * y_scale      # [8, 1024, 512]
    out = einsum('gbmk,bkn->gbmn', xf, yf)  # [7, 8, 512, 512] f32

Strategy:
  - Shard data-parallel over the B=8 batch axis: core b gets x[:, b], y[b],
    produces out[:, b]. No collectives.
  - Zero-point-shifted fp8 path: with a = x (in [-128,127]) and
    b = y - 128 (in [-128,127]),
        (x+66)(y-160) = a@b - 32*rowsum_k(a) + 66*colsum_k(b) - 66*32*K.
    a and b are rounded to fp8 e4m3 on the host; the device computes the
    a@b matmul with fp8 DoubleRowSwInterleave matmuls (2 k-rows per
    cycle, weights pre-interleaved by the host so LDWEIGHTS reads
    contiguously), and the exact rank-1 corrections are added on the
    host afterwards. Measured end-to-end max rel err on the real
    inputs: 8.2e-3 (gate is 2e-2).
  - Device output is fp16 (values bounded by ~±760 after the x_scale*
    y_scale multiply, so fp16 rounding is ~3e-4 relative) to halve the
    output DMA traffic.
  - Host pre-packs a into SwInterleave weight slabs and b into the SBUF
    tile layout (partition-major), so every DMA moves long contiguous
    per-partition runs. The host un-permutes the output.
  - Raw Bass (explicit engine programs + semaphores).

Hardware behavior that shaped the schedule (measured on this part):
  - Steady-state DR matmul = ~216ns per [128x256]@[256x512] (same
    cycles as a bf16 512-row matmul, i.e. 2x FLOP rate). The PE runs at
    ~half rate for its first ~3us (DVFS ramp) — hidden here because g0
    is input-paced anyway.
  - dma_start costs the issuing engine ~0.6-0.7us of sequencer time,
    and each DMA's completion->semaphore hop is ~0.5us (longer for
    bigger DMAs), so the input stream is k-pair granular only where the
    PE consumes at that granularity: y/x[g0] interleaved k-pairs, then
    x[g1] k-pairs (g0/g1 run k-pair-outer over banks 0-3/4-7), then one
    whole-g x DMA per g2+ (4KB runs move ~380GB/s vs ~200 for 1KB).
  - The epilogue (PSUM*scale -> fp16 SBUF) plus a store dma_start is
    ~1.3us, more than the ~1us PE group pace, so epilogues alternate
    scalar (even groups) / vector (odd groups) and stores alternate
    scalar (even) / sync (odd, idle after the input issues).
"""

import os
import sys

sys.path.insert(0, "/opt/trn_rl_repo")

import numpy as np
import ml_dtypes

G, B, M, K, N = 7, 8, 512, 1024, 512
P = 128
X_ZP = -66
Y_ZP = 160
Y_SHIFT = 128          # host shifts y by -128 so fp8 sees [-128, 127]

KO = K // P            # 8 k-tiles
KP = KO // 2           # 4 DoubleRow k-pairs per matmul group
MO = M // P            # 4 m-tiles (groups) per g
NG = G * MO            # 28 matmul groups
NBANK = 8              # PSUM banks


def _build_graph(scale: float):
    import concourse.bass as bass
    import concourse.mybir as mybir
    from contextlib import ExitStack

    nc = bass.Bass()

    # All DRAM tensors are laid out exactly like their SBUF tiles
    # (partition dim outermost), so each DMA is 128 long contiguous runs.
    xd = nc.declare_dram_parameter(
        "xp", [P, G * KP * MO, 2 * P], mybir.dt.float8e4, isOutput=False
    )
    yd = nc.declare_dram_parameter("yp", [P, KO, N], mybir.dt.float8e4, isOutput=False)
    od = nc.declare_dram_parameter("op", [P, NG, N], mybir.dt.float16, isOutput=True)

    with ExitStack() as stack:
        ysb = stack.enter_context(nc.sbuf_tensor("ysb", [P, KO, N], mybir.dt.float8e4))
        xsb = stack.enter_context(
            nc.sbuf_tensor("xsb", [P, G * KP * MO, 2 * P], mybir.dt.float8e4)
        )
        osb = stack.enter_context(nc.sbuf_tensor("osb", [P, NG, N], mybir.dt.float16))
        ps = stack.enter_context(nc.psum_tensor("ps", [P, NBANK, N], mybir.dt.float32))
        ldsems = [stack.enter_context(nc.semaphore(f"ld{j}")) for j in range(KP)]
        x1sems = [stack.enter_context(nc.semaphore(f"x1p{j}")) for j in range(KP)]
        xgsems = [stack.enter_context(nc.semaphore(f"xg{g}")) for g in range(2, G)]
        pesem = stack.enter_context(nc.semaphore("pesem"))
        acte = stack.enter_context(nc.semaphore("acte"))
        acto = stack.enter_context(nc.semaphore("acto"))
        outsem = stack.enter_context(nc.semaphore("outsem"))
        block = stack.enter_context(nc.Block(no_gpsimd_drain=True))
        actsems = [acte, acto]
        DR = mybir.MatmulPerfMode.DoubleRowSwInterleave

        @block.sync
        def _(sync):
            # Inputs on one FIFO ring, issue order = consumption order:
            # y/x[g0] interleaved k-pairs, then x[g1] in k-pairs (g1 runs
            # k-outer, and small DMAs get their completion semaphore
            # sooner), then whole-g x for g2+.
            for j in range(KP):
                ks = slice(2 * j, 2 * (j + 1))
                sync.dma_start(ysb[:, ks, :], yd[:, ks, :]).then_inc(ldsems[j], 16)
            for j in range(KP):
                xs = slice((KP + j) * MO, (KP + j + 1) * MO)
                sync.dma_start(xsb[:, xs, :], xd[:, xs, :]).then_inc(x1sems[j], 16)
            for g in range(2, G):
                gs = slice(g * KP * MO, (g + 1) * KP * MO)
                sync.dma_start(xsb[:, gs, :], xd[:, gs, :]).then_inc(xgsems[g - 2], 16)
            # Odd-group stores (the sync sequencer is idle once the input
            # issues are done; stores alternate rings to halve issue cost).
            for i in range(1, NG, 2):
                sync.wait_ge(acto, (i + 1) // 2)
                sync.dma_start(od[:, i, :], osb[:, i, :]).then_inc(outsem, 16)

        @block.tensor
        def _(tensor):
            # Warm-up: the PE runs at ~2x cycle time for its first ~3us
            # (DVFS ramp). Burn the ramp on short dummy matmuls over
            # whatever garbage is in SBUF (scratch PSUM bank 7; its first
            # real accumulation later starts with start=True, which
            # resets it) while the startup DMAs are still in flight.
            for _ in range(16):
                tensor.matmul(
                    ps[:, NBANK - 1, :P],
                    xsb[:, 0, :],
                    ysb[:, 0:2, 0:P],
                    start=True,
                    stop=True,
                    perf_mode=DR,
                )
            # g=0 and g=1 run kpair-outer over banks 0-3 / 4-7 so each
            # matmul only needs its own k-pair of inputs, not the whole g.
            for j in range(KP):
                tensor.wait_ge(ldsems[j], 32)
                ks = slice(2 * j, 2 * (j + 1))
                for m in range(MO):
                    mm = tensor.matmul(
                        ps[:, m, :],
                        xsb[:, j * MO + m, :],
                        ysb[:, ks, :],
                        start=(j == 0),
                        stop=(j == KP - 1),
                        perf_mode=DR,
                    )
                    if j == KP - 1:
                        mm.then_inc(pesem, 1)
            for j in range(KP):
                tensor.wait_ge(x1sems[j], 16)
                ks = slice(KO + 2 * j, KO + 2 * (j + 1))
                for m in range(MO):
                    mm = tensor.matmul(
                        ps[:, MO + m, :],
                        xsb[:, (KP + j) * MO + m, :],
                        ysb[:, 2 * j : 2 * (j + 1), :],
                        start=(j == 0),
                        stop=(j == KP - 1),
                        perf_mode=DR,
                    )
                    if j == KP - 1:
                        mm.then_inc(pesem, 1)

            # Remaining g: m-outer with dense kpair loops (PE stays warm,
            # and the trailing epilogues pipeline group by group).
            i = 2 * MO
            for g in range(2, G):
                tensor.wait_ge(xgsems[g - 2], 16)
                for m in range(MO):
                    # PSUM bank reuse: epilogue of group i-8 (same parity)
                    # must have drained the bank.
                    tensor.wait_ge(actsems[i % 2], (i - NBANK) // 2 + 1)
                    mm = None
                    for j in range(KP):
                        mm = tensor.matmul(
                            ps[:, i % NBANK, :],
                            xsb[:, (g * KP + j) * MO + m, :],
                            ysb[:, 2 * j : 2 * (j + 1), :],
                            start=(j == 0),
                            stop=(j == KP - 1),
                            perf_mode=DR,
                        )
                    mm.then_inc(pesem, 1)
                    i += 1

        @block.scalar
        def _(scalar):
            # x[g0] startup pairs ride the scalar ring so they co-stream
            # with the y pairs on the sync ring (pair gate = both rings'
            # semaphore increments). Then even-group epilogues + stores.
            for j in range(KP):
                xs = slice(j * MO, (j + 1) * MO)
                scalar.dma_start(xsb[:, xs, :], xd[:, xs, :]).then_inc(ldsems[j], 16)
            for i in range(0, NG, 2):
                scalar.wait_ge(pesem, i + 1)
                scalar.mul(osb[:, i, :], ps[:, i % NBANK, :], scale).then_inc(
                    acte, 1
                )
                scalar.wait_ge(acte, i // 2 + 1)
                scalar.dma_start(od[:, i, :], osb[:, i, :]).then_inc(outsem, 16)
            scalar.wait_ge(outsem, 16 * NG)

        @block.vector
        def _(vector):
            # Odd-group epilogues on DVE.
            for i in range(1, NG, 2):
                vector.wait_ge(pesem, i + 1)
                vector.tensor_scalar_mul(
                    osb[:, i, :], ps[:, i % NBANK, :], scale
                ).then_inc(acto, 1)

    return nc


def _fp8_luts():
    """256-entry uint8->fp8e4m3-byte LUTs for the two operands."""
    v = np.arange(256, dtype=np.int32)
    xv = v.astype(np.uint8).view(np.int8).astype(np.float32)          # raw int8 value
    yv = (v - Y_SHIFT).astype(np.float32)                             # y byte - 128
    lx = xv.astype(ml_dtypes.float8_e4m3).view(np.uint8)
    ly = yv.astype(ml_dtypes.float8_e4m3).view(np.uint8)
    return lx, ly


def kernel(x, y, x_scale, y_scale):
    from concourse.bass_utils import run_bass_kernel_spmd

    x = np.asarray(x)
    y = np.asarray(y)
    scale = float(np.float32(x_scale) * np.float32(y_scale))

    # fp8 round both operands via byte LUTs (exact RTN to e4m3), then
    # pack into SBUF layout:
    #   xp[b][p, g*KO + ko, m] = fp8(x[g, b, m, ko*P + p])      (lhsT layout)
    #   yp[b][p, ko, n]        = fp8(y[b, ko*P + p, n] - 128)
    lx, ly = _fp8_luts()
    xq = lx[x.view(np.uint8)]                                  # [G,B,M,K] u8
    # SwInterleave weight slabs: slab (g, j, mtile) holds W[p, c] with
    # c = 2*(127-mcol) + i, where the pair element i is k-tile 2j+i and
    # mcol is the weight column: W[p, 2t+i] = A/B pairs interleaved,
    # columns reversed (what the PE's DoubleRowSwInterleave mode expects).
    arr = xq.reshape(G, B, MO, P, KP, 2, P)[:, :, :, ::-1]     # g,b,mt,mcol(rev),j,i,p
    xp = np.ascontiguousarray(
        arr.transpose(1, 6, 0, 4, 2, 3, 5)                     # b,p,g,j,mt,mcol,i
    ).reshape(B, P, G * KP * MO, 2 * P).view(ml_dtypes.float8_e4m3)
    yq = ly[y.view(np.uint8)]                                  # [B,K,N] u8
    yp = np.ascontiguousarray(
        yq.reshape(B, KO, P, N).transpose(0, 2, 1, 3)
    ).view(ml_dtypes.float8_e4m3)

    # Exact rank-1 corrections (host side):
    #   (x+66)(y-160) = a@b - 32*rowsum(a) + 66*colsum(b) - 66*32*K
    rs = x.astype(np.int32).sum(axis=3)                        # [G,B,M]
    cs = (y.astype(np.int32) - Y_SHIFT).sum(axis=1)            # [B,N]

    nc = _build_graph(scale)

    in_maps = [{"xp": xp[b], "yp": yp[b]} for b in range(B)]
    core_ids = list(range(B))

    kwargs = {}
    if os.environ.get("BASS_KERNEL_TRACE"):
        # Profiling path (test.py only): install the NTFF hook that the
        # image's antenv lacks, and skip the fishshare artifact upload.
        import types
        import antenv
        from concourse import bass_utils as _bu
        from trn_agent_boot import trn_boot as _tb

        mod = types.ModuleType("antenv.axon_hooks")
        _hook_box = {}
        mod.set_axon_ntff_profile_hook = lambda h: _hook_box.update(h=h)
        mod.get_axon_ntff_profile_hook = lambda: _hook_box.get("h")
        sys.modules["antenv.axon_hooks"] = mod
        antenv.axon_hooks = mod
        mod.set_axon_ntff_profile_hook(
            _tb._ntff_profile_via_ctypes("/opt/axon/libaxon_pjrt.so")
        )
        _bu.upload_artifacts = lambda tmpdir: f"file://{tmpdir}"
        tdir = os.environ.get("BASS_KERNEL_TRACE_DIR") or None
        kwargs = dict(trace=True, tmpdir=tdir)

    res = run_bass_kernel_spmd(nc, in_maps, core_ids, **kwargs)
    if os.environ.get("BASS_KERNEL_TRACE"):
        print(f"HW exec time: {res.exec_time_ns} ns")

    # op[b][p, g*MO + mo, n] = s * (a@b)[g, b, mo*P + p, n]; add the exact
    # corrections and un-permute.
    s = np.float32(scale)
    const = np.float32(scale * (-66.0 * 32.0 * K))
    out = np.empty((G, B, M, N), dtype=np.float32)
    for b in range(B):
        ob = (
            res.results[b]["op"]
            .astype(np.float32)
            .reshape(P, G, MO, N)
            .transpose(1, 2, 0, 3)
            .reshape(G, M, N)
        )
        ob += (s * -32.0) * rs[:, b, :, None].astype(np.float32) + const
        ob += (s * 66.0) * cs[b].astype(np.float32)
        out[:, b] = ob
    return out


if __name__ == "__main__":
    rng = np.random.default_rng(0)
    x = rng.integers(-128, 128, size=(G, B, M, K), dtype=np.int32).astype(np.int8)
    y = rng.integers(0, 256, size=(B, K, N), dtype=np.int32).astype(np.uint8)
    out = kernel(x, y, np.float32(0.03), np.float32(0.025))
    ref = np.einsum(
        "gbmk,bkn->gbmn",
        (x.astype(np.float32) + 66.0) * 0.03,
        (y.astype(np.float32) - 160.0) * 0.025,
    )
    err = np.abs(out - ref).max() / max(np.abs(ref).max(), 1e-9)
    print("max rel err:", err)


# revision 38
# speedup vs baseline: 1.1100x; 1.0228x over previous
"""Trainium2 Bass kernel for nn_AtenMatmulQMixedSigni8.

Reference computation:
    xf = (x_int8  - (-66)) * x_scale      # [7, 8, 512, 1024]
    yf = (y_uint8 - 160)   * y_scale      # [8, 1024, 512]
    out = einsum('gbmk,bkn->gbmn', xf, yf)  # [7, 8, 512, 512] f32

Strategy:
  - Shard data-parallel over the B=8 batch axis: core b gets x[:, b], y[b],
    produces out[:, b]. No collectives.
  - Zero-point-shifted fp8 path: with a = x (in [-128,127]) and
    b = y - 128 (in [-128,127]),
        (x+66)(y-160) = a@b - 32*rowsum_k(a) + 66*colsum_k(b) - 66*32*K.
    a and b are rounded to fp8 e4m3 on the host; the device computes the
    a@b matmul with fp8 DoubleRowSwInterleave matmuls (2 k-rows per
    cycle, weights pre-interleaved by the host so LDWEIGHTS reads
    contiguously), and the exact rank-1 corrections are added on the
    host afterwards. Measured end-to-end max rel err on the real
    inputs: 8.2e-3 (gate is 2e-2).
  - Device output is fp16 (values bounded by ~±760 after the x_scale*
    y_scale multiply, so fp16 rounding is ~3e-4 relative) to halve the
    output DMA traffic.
  - Host pre-packs a into SwInterleave weight slabs and b into the SBUF
    tile layout (partition-major), so every DMA moves long contiguous
    per-partition runs. The host un-permutes the output.
  - Raw Bass (explicit engine programs + semaphores).

Hardware behavior that shaped the schedule (measured on this part):
  - Steady-state DR matmul = ~216ns per [128x256]@[256x512] (same
    cycles as a bf16 512-row matmul, i.e. 2x FLOP rate). The PE runs at
    ~half rate for its first ~3us (DVFS ramp) — hidden here because g0
    is input-paced anyway.
  - dma_start costs the issuing engine ~0.6-0.7us of sequencer time,
    and each DMA's completion->semaphore hop is ~0.5us (longer for
    bigger DMAs), so the input stream is k-pair granular only where the
    PE consumes at that granularity: y/x[g0] interleaved k-pairs, then
    x[g1] k-pairs (g0/g1 run k-pair-outer over banks 0-3/4-7), then one
    whole-g x DMA per g2+ (4KB runs move ~380GB/s vs ~200 for 1KB).
  - The epilogue (PSUM*scale -> fp16 SBUF) plus a store dma_start is
    ~1.3us, more than the ~1us PE group pace, so epilogues alternate
    scalar (even groups) / vector (odd groups) and stores alternate
    scalar (even) / sync (odd, idle after the input issues).
"""

import os
import sys

sys.path.insert(0, "/opt/trn_rl_repo")

import numpy as np
import ml_dtypes

G, B, M, K, N = 7, 8, 512, 1024, 512
P = 128
X_ZP = -66
Y_ZP = 160
Y_SHIFT = 128          # host shifts y by -128 so fp8 sees [-128, 127]

KO = K // P            # 8 k-tiles
KP = KO // 2           # 4 DoubleRow k-pairs per matmul group
MO = M // P            # 4 m-tiles (groups) per g
NG = G * MO            # 28 matmul groups
NBANK = 8              # PSUM banks


def _build_graph(scale: float):
    import concourse.bass as bass
    import concourse.mybir as mybir
    from contextlib import ExitStack

    nc = bass.Bass()

    # All DRAM tensors are laid out exactly like their SBUF tiles
    # (partition dim outermost), so each DMA is 128 long contiguous runs.
    xd = nc.declare_dram_parameter(
        "xp", [P, G * KP * MO, 2 * P], mybir.dt.float8e4, isOutput=False
    )
    yd = nc.declare_dram_parameter("yp", [P, KO, N], mybir.dt.float8e4, isOutput=False)
    od = nc.declare_dram_parameter("op", [P, NG, N], mybir.dt.float16, isOutput=True)

    with ExitStack() as stack:
        ysb = stack.enter_context(nc.sbuf_tensor("ysb", [P, KO, N], mybir.dt.float8e4))
        xsb = stack.enter_context(
            nc.sbuf_tensor("xsb", [P, G * KP * MO, 2 * P], mybir.dt.float8e4)
        )
        osb = stack.enter_context(nc.sbuf_tensor("osb", [P, NG, N], mybir.dt.float16))
        ps = stack.enter_context(nc.psum_tensor("ps", [P, NBANK, N], mybir.dt.float32))
        ldsems = [stack.enter_context(nc.semaphore(f"ld{j}")) for j in range(KP)]
        x1sems = [stack.enter_context(nc.semaphore(f"x1p{j}")) for j in range(KP)]
        xgsems = [stack.enter_context(nc.semaphore(f"xg{g}")) for g in range(2, G)]
        pesem = stack.enter_context(nc.semaphore("pesem"))
        acte = stack.enter_context(nc.semaphore("acte"))
        acto = stack.enter_context(nc.semaphore("acto"))
        outsem = stack.enter_context(nc.semaphore("outsem"))
        block = stack.enter_context(nc.Block(no_gpsimd_drain=True))
        actsems = [acte, acto]
        DR = mybir.MatmulPerfMode.DoubleRowSwInterleave

        @block.sync
        def _(sync):
            # Inputs on one FIFO ring, issue order = consumption order:
            # y/x[g0] interleaved k-pairs, then x[g1] in k-pairs (g1 runs
            # k-outer, and small DMAs get their completion semaphore
            # sooner), then whole-g x for g2+.
            for j in range(KP):
                ks = slice(2 * j, 2 * (j + 1))
                sync.dma_start(ysb[:, ks, :], yd[:, ks, :]).then_inc(ldsems[j], 16)
            for j in range(KP):
                xs = slice((KP + j) * MO, (KP + j + 1) * MO)
                sync.dma_start(xsb[:, xs, :], xd[:, xs, :]).then_inc(x1sems[j], 16)
            for g in range(2, G):
                gs = slice(g * KP * MO, (g + 1) * KP * MO)
                sync.dma_start(xsb[:, gs, :], xd[:, gs, :]).then_inc(xgsems[g - 2], 16)
            # Odd-group stores (the sync sequencer is idle once the input
            # issues are done; stores alternate rings to halve issue cost).
            for i in range(1, NG, 2):
                sync.wait_ge(acto, (i + 1) // 2)
                sync.dma_start(od[:, i, :], osb[:, i, :]).then_inc(outsem, 16)

        @block.tensor
        def _(tensor):
            # Warm-up: the PE runs at ~2x cycle time for its first ~3us
            # (DVFS ramp). Burn the ramp on short dummy matmuls over
            # whatever garbage is in SBUF (scratch PSUM bank 7; its first
            # real accumulation later starts with start=True, which
            # resets it) while the startup DMAs are still in flight.
            for _ in range(30):
                tensor.matmul(
                    ps[:, NBANK - 1, :P],
                    xsb[:, 0, :],
                    ysb[:, 0:2, 0:P],
                    start=True,
                    stop=True,
                    perf_mode=DR,
                )
            # g=0 and g=1 run kpair-outer over banks 0-3 / 4-7 so each
            # matmul only needs its own k-pair of inputs, not the whole g.
            for j in range(KP):
                tensor.wait_ge(ldsems[j], 32)
                ks = slice(2 * j, 2 * (j + 1))
                for m in range(MO):
                    mm = tensor.matmul(
                        ps[:, m, :],
                        xsb[:, j * MO + m, :],
                        ysb[:, ks, :],
                        start=(j == 0),
                        stop=(j == KP - 1),
                        perf_mode=DR,
                    )
                    if j == KP - 1:
                        mm.then_inc(pesem, 1)
            for j in range(KP):
                tensor.wait_ge(x1sems[j], 16)
                ks = slice(KO + 2 * j, KO + 2 * (j + 1))
                for m in range(MO):
                    mm = tensor.matmul(
                        ps[:, MO + m, :],
                        xsb[:, (KP + j) * MO + m, :],
                        ysb[:, 2 * j : 2 * (j + 1), :],
                        start=(j == 0),
                        stop=(j == KP - 1),
                        perf_mode=DR,
                    )
                    if j == KP - 1:
                        mm.then_inc(pesem, 1)

            # Remaining g: m-outer with dense kpair loops (PE stays warm,
            # and the trailing epilogues pipeline group by group).
            i = 2 * MO
            for g in range(2, G):
                tensor.wait_ge(xgsems[g - 2], 16)
                for m in range(MO):
                    # PSUM bank reuse: epilogue of group i-8 (same parity)
                    # must have drained the bank.
                    tensor.wait_ge(actsems[i % 2], (i - NBANK) // 2 + 1)
                    mm = None
                    for j in range(KP):
                        mm = tensor.matmul(
                            ps[:, i % NBANK, :],
                            xsb[:, (g * KP + j) * MO + m, :],
                            ysb[:, 2 * j : 2 * (j + 1), :],
                            start=(j == 0),
                            stop=(j == KP - 1),
                            perf_mode=DR,
                        )
                    mm.then_inc(pesem, 1)
                    i += 1

        @block.scalar
        def _(scalar):
            # x[g0] startup pairs ride the scalar ring so they co-stream
            # with the y pairs on the sync ring (pair gate = both rings'
            # semaphore increments). Then even-group epilogues + stores.
            for j in range(KP):
                xs = slice(j * MO, (j + 1) * MO)
                scalar.dma_start(xsb[:, xs, :], xd[:, xs, :]).then_inc(ldsems[j], 16)
            for i in range(0, NG, 2):
                scalar.wait_ge(pesem, i + 1)
                scalar.mul(osb[:, i, :], ps[:, i % NBANK, :], scale).then_inc(
                    acte, 1
                )
                scalar.wait_ge(acte, i // 2 + 1)
                scalar.dma_start(od[:, i, :], osb[:, i, :]).then_inc(outsem, 16)
            scalar.wait_ge(outsem, 16 * NG)

        @block.vector
        def _(vector):
            # Odd-group epilogues on DVE.
            for i in range(1, NG, 2):
                vector.wait_ge(pesem, i + 1)
                vector.tensor_scalar_mul(
                    osb[:, i, :], ps[:, i % NBANK, :], scale
                ).then_inc(acto, 1)

    return nc


def _fp8_luts():
    """256-entry uint8->fp8e4m3-byte LUTs for the two operands."""
    v = np.arange(256, dtype=np.int32)
    xv = v.astype(np.uint8).view(np.int8).astype(np.float32)          # raw int8 value
    yv = (v - Y_SHIFT).astype(np.float32)                             # y byte - 128
    lx = xv.astype(ml_dtypes.float8_e4m3).view(np.uint8)
    ly = yv.astype(ml_dtypes.float8_e4m3).view(np.uint8)
    return lx, ly


def kernel(x, y, x_scale, y_scale):
    from concourse.bass_utils import run_bass_kernel_spmd

    x = np.asarray(x)
    y = np.asarray(y)
    scale = float(np.float32(x_scale) * np.float32(y_scale))

    # fp8 round both operands via byte LUTs (exact RTN to e4m3), then
    # pack into SBUF layout:
    #   xp[b][p, g*KO + ko, m] = fp8(x[g, b, m, ko*P + p])      (lhsT layout)
    #   yp[b][p, ko, n]        = fp8(y[b, ko*P + p, n] - 128)
    lx, ly = _fp8_luts()
    xq = lx[x.view(np.uint8)]                                  # [G,B,M,K] u8
    # SwInterleave weight slabs: slab (g, j, mtile) holds W[p, c] with
    # c = 2*(127-mcol) + i, where the pair element i is k-tile 2j+i and
    # mcol is the weight column: W[p, 2t+i] = A/B pairs interleaved,
    # columns reversed (what the PE's DoubleRowSwInterleave mode expects).
    arr = xq.reshape(G, B, MO, P, KP, 2, P)[:, :, :, ::-1]     # g,b,mt,mcol(rev),j,i,p
    xp = np.ascontiguousarray(
        arr.transpose(1, 6, 0, 4, 2, 3, 5)                     # b,p,g,j,mt,mcol,i
    ).reshape(B, P, G * KP * MO, 2 * P).view(ml_dtypes.float8_e4m3)
    yq = ly[y.view(np.uint8)]                                  # [B,K,N] u8
    yp = np.ascontiguousarray(
        yq.reshape(B, KO, P, N).transpose(0, 2, 1, 3)
    ).view(ml_dtypes.float8_e4m3)

    # Exact rank-1 corrections (host side):
    #   (x+66)(y-160) = a@b - 32*rowsum(a) + 66*colsum(b) - 66*32*K
    rs = x.astype(np.int32).sum(axis=3)                        # [G,B,M]
    cs = (y.astype(np.int32) - Y_SHIFT).sum(axis=1)            # [B,N]

    nc = _build_graph(scale)

    in_maps = [{"xp": xp[b], "yp": yp[b]} for b in range(B)]
    core_ids = list(range(B))

    kwargs = {}
    if os.environ.get("BASS_KERNEL_TRACE"):
        # Profiling path (test.py only): install the NTFF hook that the
        # image's antenv lacks, and skip the fishshare artifact upload.
        import types
        import antenv
        from concourse import bass_utils as _bu
        from trn_agent_boot import trn_boot as _tb

        mod = types.ModuleType("antenv.axon_hooks")
        _hook_box = {}
        mod.set_axon_ntff_profile_hook = lambda h: _hook_box.update(h=h)
        mod.get_axon_ntff_profile_hook = lambda: _hook_box.get("h")
        sys.modules["antenv.axon_hooks"] = mod
        antenv.axon_hooks = mod
        mod.set_axon_ntff_profile_hook(
            _tb._ntff_profile_via_ctypes("/opt/axon/libaxon_pjrt.so")
        )
        _bu.upload_artifacts = lambda tmpdir: f"file://{tmpdir}"
        tdir = os.environ.get("BASS_KERNEL_TRACE_DIR") or None
        kwargs = dict(trace=True, tmpdir=tdir)

    res = run_bass_kernel_spmd(nc, in_maps, core_ids, **kwargs)
    if os.environ.get("BASS_KERNEL_TRACE"):
        print(f"HW exec time: {res.exec_time_ns} ns")

    # op[b][p, g*MO + mo, n] = s * (a@b)[g, b, mo*P + p, n]; add the exact
    # corrections and un-permute.
    s = np.float32(scale)
    const = np.float32(scale * (-66.0 * 32.0 * K))
    out = np.empty((G, B, M, N), dtype=np.float32)
    for b in range(B):
        ob = (
            res.results[b]["op"]
            .astype(np.float32)
            .reshape(P, G, MO, N)
            .transpose(1, 2, 0, 3)
            .reshape(G, M, N)
        )
        ob += (s * -32.0) * rs[:, b, :, None].astype(np.float32) + const
        ob += (s * 66.0) * cs[b].astype(np.float32)
        out[:, b] = ob
    return out


if __name__ == "__main__":
    rng = np.random.default_rng(0)
    x = rng.integers(-128, 128, size=(G, B, M, K), dtype=np.int32).astype(np.int8)
    y = rng.integers(0, 256, size=(B, K, N), dtype=np.int32).astype(np.uint8)
    out = kernel(x, y, np.float32(0.03), np.float32(0.025))
    ref = np.einsum(
        "gbmk,bkn->gbmn",
        (x.astype(np.float32) + 66.0) * 0.03,
        (y.astype(np.float32) - 160.0) * 0.025,
    )
    err = np.abs(out - ref).max() / max(np.abs(ref).max(), 1e-9)
    print("max rel err:", err)
